# revision 9
# baseline (speedup 1.0000x reference)
"""Trainium2 Bass kernel for the AmbiguityHead (retrieval_knn) problem.

Reference computation (per point i, K=15 neighbors j = nidx[i,k]):
    center_cls = argmax(labels[i])          (first occurrence on ties)
    neigh_cls  = argmax(labels[j])
    posmask    = center_cls == neigh_cls
    d2         = ||p[i] - p[j]||^2
    w          = softmax(-CCBETA * d2 over k)
    out[i]     = NU * sum_k w_k * (1 - posmask_k)

Strategy (8 NeuronCores, data-parallel over points; see spec sharding_hint):
  - Each core owns 62,500 points (padded to 63,488 = 128*496, laid out
    point-interleaved: point n -> partition n%128, slot n//128).
  - Phase 1: per-shard argmax class (encoded 13-argmax to keep jnp first-
    occurrence tie semantics), pack 16B records (x,y,z,clsenc), AllGather
    the packed table (8 x 63,488 rows) into pair-shared HBM.
  - Phase 2: the per-pair random gather uses the custom InstDMAGatherAnt
    ucode op: 256B super-rows (16 records) indexed by int16 j>>4 (the
    permuted table has 31,744 super-rows < 32768), issued over the 4 SWDGE
    queues. The 16B record is then extracted on-chip with a 4-level binary
    select tree keyed on bits of j&15, followed by the distance/softmax/
    compare reduction on the Vector/Scalar engines.
  - Host side does only sharding/layout: row permutations, index
    re-encoding (j -> [permuted row]>>4 / &15, int16 wrapped layout), and
    inverse permutation of the output. All arithmetic of the reference
    (argmax, distances, exp, reductions, compares) runs on device.
"""

import numpy as np

import concourse.bass as bass
import concourse.mybir as mybir
import concourse.tile as tile
from concourse import bacc
from concourse import bass_utils

F32 = mybir.dt.float32
I32 = mybir.dt.int32
I16 = mybir.dt.int16
I8 = mybir.dt.int8
AX = mybir.AxisListType
OP = mybir.AluOpType

N_CORES = 8
P = 128
C = 13
K = 15
CCBETA = 2.0
NU = 1.0

FULL_S = 500_000 // N_CORES   # real points per core
FULL_G = 496                  # point slots per partition (128*496 = 63488)
B = 4                         # point-groups (of 128) per gather call
NIDX_CALL = P * K * B         # 7680 indices per gather call
NW_CALL = NIDX_CALL // 16     # 480 idx per partition line (wrapped int16)
SLOTS = B * K                 # 60 pair slots per partition per call
DMA_SCRATCH = 65536
N_QUEUES = 4


def raw_dma_gather(nc, out_ap, in_ap, idxs_ap, num_idxs, elem_size, elem_step,
                   queue_num):
    """bass.dma_gather for elem_size*dtype % 256 != 0 (the ucode only needs
    the stride to be a 256B multiple for non-transpose HBM gathers; the
    bass-level assert is transpose-only in the ucode)."""
    eng = nc.gpsimd
    stride_bytes = elem_step * mybir.dt.size(in_ap.dtype)
    stride_bytes_256 = stride_bytes // 256
    assert stride_bytes % 256 == 0 and 0 < stride_bytes_256 < 256
    _in_ap = eng.lower_ap_dma(in_ap, for_custom_bir_dma=True)
    _idxs_ap = eng.lower_ap(idxs_ap)
    _out_ap = eng.lower_ap(out_ap)
    return eng.add_instruction(
        mybir.InstDMAGatherAnt(
            name=nc.get_next_instruction_name(),
            ins=[*_in_ap, _idxs_ap, eng.lower_val_access(eng.to_reg(num_idxs))],
            outs=[_out_ap],
            transpose=False,
            num_idxs=num_idxs,
            elem_size=elem_size,
            stride_bytes_256=stride_bytes_256,
            gen_mode=0,
            single_packet=False,
            queue_num=queue_num,
            sbuf_tokens_per_rank=0,
            sbuf_free_dim_per_rank=0,
            sbuf_free_dim_pad_per_rank=0,
            sbuf_byte_offset=0,
        ))


def select3(nc, out, mask, on_true, on_false):
    """nc.vector.select with un-merged APs so all three operands keep the
    same 3D shape (the stock path flattens only the contiguous out AP)."""
    v = nc.vector
    v.tensor_copy(out, on_false)
    return v.add_instruction(
        mybir.InstCopyPredicated(
            name=nc.get_next_instruction_name(),
            ins=[v.lower_ap(mask, opt=False), v.lower_ap(on_true, opt=False)],
            outs=[v.lower_ap(out, opt=False)],
        ))


def build(S=FULL_S, G=FULL_G):
    SP = P * G
    assert SP >= S and G % B == 0
    CALLS = G // B
    NTAB = N_CORES * SP          # permuted global table rows
    assert NTAB // 16 <= 32767   # int16 super-row indices

    nc = bacc.Bacc("TRN2", target_bir_lowering=False, debug=False,
                   num_devices=N_CORES, dynamic_dma_scratch_size=DMA_SCRATCH,
                   num_swdge_queues=N_QUEUES)

    # host-permuted inputs: row p*G+g holds point g*128+p of this core
    labels = nc.dram_tensor("labels", [SP, C], F32, kind="ExternalInput")
    p3 = nc.dram_tensor("p3", [SP, 3], F32, kind="ExternalInput")
    # per call: wrapped int16 super-row idxs [128, NW_CALL] and int8 a=j&15
    # in gather-slot layout [128, SLOTS]
    qidx = nc.dram_tensor("qidx", [CALLS, P, NW_CALL], I16, kind="ExternalInput")
    aidx = nc.dram_tensor("aidx", [CALLS, P, SLOTS], I8, kind="ExternalInput")
    out = nc.dram_tensor("out", [P, G], F32, kind="ExternalOutput")

    tshard = nc.dram_tensor("tshard", [SP, 4], F32)
    table_b = nc.dram_tensor("table_b", [NTAB, 4], F32, addr_space="Shared")
    tab64 = table_b.ap().rearrange("(q s) c -> q (s c)", s=16)  # [NTAB/16, 64]

    with tile.TileContext(nc) as tc:
        with tc.tile_pool(name="pers", bufs=1) as pers:
            # ---------------- phase 1: packed class/coord table ----------------
            with tc.tile_pool(name="ph1", bufs=1) as ph1:
                lab = ph1.tile([P, G * C], F32)
                nc.sync.dma_start(
                    out=lab[:], in_=labels.ap().rearrange("(p g) c -> p (g c)", p=P))
                lab3 = lab[:].rearrange("p (g c) -> p g c", c=C)

                m = ph1.tile([P, G], F32)
                nc.vector.tensor_reduce(m[:].unsqueeze(2), lab3, AX.X, op=OP.max)

                revio_i = ph1.tile([P, C], I32)
                nc.gpsimd.iota(revio_i[:], pattern=[[-1, C]], base=C,
                               channel_multiplier=0)
                revio = ph1.tile([P, C], F32)
                nc.vector.tensor_copy(revio[:], revio_i[:])

                eq = ph1.tile([P, G * C], F32)
                eq3 = eq[:].rearrange("p (g c) -> p g c", c=C)
                nc.vector.tensor_tensor(
                    eq3, lab3, m[:].unsqueeze(2).broadcast_to((P, G, C)),
                    OP.is_equal)
                nc.vector.tensor_tensor(
                    eq3, eq3, revio[:].unsqueeze(1).broadcast_to((P, G, C)),
                    OP.mult)
                cls = pers.tile([P, G], F32)
                nc.vector.tensor_reduce(cls[:].unsqueeze(2), eq3, AX.X, op=OP.max)

                pt = pers.tile([P, G * 3], F32)
                nc.sync.dma_start(
                    out=pt[:], in_=p3.ap().rearrange("(p g) c -> p (g c)", p=P))
                pt3 = pt[:].rearrange("p (g c) -> p g c", c=3)

                pk = ph1.tile([P, G * 4], F32)
                pk3 = pk[:].rearrange("p (g c) -> p g c", c=4)
                nc.vector.tensor_copy(pk3[:, :, 0:3], pt3)
                nc.vector.tensor_copy(pk3[:, :, 3:4], cls[:].unsqueeze(2))

                nc.sync.dma_start(
                    out=tshard.ap().rearrange("(p g) c -> p (g c)", p=P),
                    in_=pk[:])

                nc.gpsimd.collective_compute(
                    "AllGather", OP.bypass,
                    replica_groups=[list(range(N_CORES))],
                    ins=[tshard.ap().opt()],
                    outs=[table_b.ap().opt()],
                )

            # ---------------- phase 2: gather + extract + reduce --------------
            out_sb = pers.tile([P, G], F32)
            with tc.tile_pool(name="io", bufs=4) as iop, \
                 tc.tile_pool(name="cmp", bufs=2) as cmp:
                for call in range(CALLS):
                    gcol = slice(call * B, (call + 1) * B)
                    qt = iop.tile([P, NW_CALL], I16, tag="qt")
                    nc.sync.dma_start(out=qt[:], in_=qidx[call, :, :])
                    at = iop.tile([P, SLOTS], I8, tag="at")
                    nc.sync.dma_start(out=at[:], in_=aidx[call, :, :])

                    gat = iop.tile([P, SLOTS * 64], F32, tag="gat")
                    nc.gpsimd.dma_gather(
                        out_ap=gat[:].rearrange("p (n e) -> p n e", e=64),
                        in_ap=tab64,
                        idxs_ap=qt[:],
                        num_idxs=NIDX_CALL,
                        num_idxs_reg=NIDX_CALL,
                        elem_size=64,
                        single_packet=False,
                        queue_num=call % N_QUEUES,
                    )
                    g3 = gat[:].rearrange("p (n e) -> p n e", e=64)

                    # bit masks of a = j & 15 (int8 0/1 for CopyPredicated),
                    # top bit first; nr tracks the negated residual.
                    masks = [None] * 4
                    m3 = cmp.tile([P, SLOTS], I8, tag="mb3")
                    nc.vector.tensor_scalar(m3[:], at[:], 8, None, OP.is_ge)
                    masks[3] = m3
                    nr = cmp.tile([P, SLOTS], I8, tag="nr")
                    nc.vector.scalar_tensor_tensor(
                        nr[:], m3[:], 8, at[:], OP.mult, OP.subtract)
                    for b in (2, 1, 0):
                        mb = cmp.tile([P, SLOTS], I8, tag=f"mb{b}")
                        nc.vector.tensor_scalar(
                            mb[:], nr[:], -(1 << b), None, OP.is_le)
                        masks[b] = mb
                        if b > 0:
                            nc.vector.scalar_tensor_tensor(
                                nr[:], mb[:], 1 << b, nr[:],
                                OP.mult, OP.add)

                    # binary select tree: pick the right 16B record of 64 f32
                    t32 = cmp.tile([P, SLOTS * 32], F32, tag="t32")
                    t32v = t32[:].rearrange("p (n e) -> p n e", e=32)
                    select3(
                        nc, t32v,
                        masks[3][:].unsqueeze(2).broadcast_to((P, SLOTS, 32)),
                        g3[:, :, 32:64], g3[:, :, 0:32])
                    t16 = cmp.tile([P, SLOTS * 16], F32, tag="t16")
                    t16v = t16[:].rearrange("p (n e) -> p n e", e=16)
                    select3(
                        nc, t16v,
                        masks[2][:].unsqueeze(2).broadcast_to((P, SLOTS, 16)),
                        t32v[:, :, 16:32], t32v[:, :, 0:16])
                    t8 = cmp.tile([P, SLOTS * 8], F32, tag="t8")
                    t8v = t8[:].rearrange("p (n e) -> p n e", e=8)
                    select3(
                        nc, t8v,
                        masks[1][:].unsqueeze(2).broadcast_to((P, SLOTS, 8)),
                        t16v[:, :, 8:16], t16v[:, :, 0:8])
                    ext = cmp.tile([P, SLOTS * 4], F32, tag="ext")
                    extv = ext[:].rearrange("p (n e) -> p n e", e=4)
                    select3(
                        nc, extv,
                        masks[0][:].unsqueeze(2).broadcast_to((P, SLOTS, 4)),
                        t8v[:, :, 4:8], t8v[:, :, 0:4])

                    # slot n = g_sub*15 + k ; component views [P, B, K]
                    def comp(cidx):
                        return ext[:].rearrange(
                            "p (g k e) -> p g k e", k=K, e=4)[:, :, :, cidx:cidx + 1
                                                             ].squeeze(3)

                    def cb(cidx):
                        return pt3[:, gcol, cidx:cidx + 1].broadcast_to((P, B, K))

                    dx = cmp.tile([P, SLOTS], F32, tag="dx")
                    dx3 = dx[:].rearrange("p (g k) -> p g k", k=K)
                    dy = cmp.tile([P, SLOTS], F32, tag="dy")
                    dy3 = dy[:].rearrange("p (g k) -> p g k", k=K)
                    dz = cmp.tile([P, SLOTS], F32, tag="dz")
                    dz3 = dz[:].rearrange("p (g k) -> p g k", k=K)
                    d2 = cmp.tile([P, SLOTS], F32, tag="d2")
                    d23 = d2[:].rearrange("p (g k) -> p g k", k=K)

                    nc.vector.tensor_tensor(dx3, comp(0), cb(0), OP.subtract)
                    nc.vector.tensor_tensor(dy3, comp(1), cb(1), OP.subtract)
                    nc.vector.tensor_tensor(dz3, comp(2), cb(2), OP.subtract)
                    nc.vector.tensor_tensor(d23, dx3, dx3, OP.mult)
                    nc.vector.tensor_tensor(dy3, dy3, dy3, OP.mult)
                    nc.vector.tensor_tensor(d23, d23, dy3, OP.add)
                    nc.vector.tensor_tensor(dz3, dz3, dz3, OP.mult)
                    nc.vector.tensor_tensor(d23, d23, dz3, OP.add)

                    neg = cmp.tile([P, SLOTS], F32, tag="neg")
                    neg3 = neg[:].rearrange("p (g k) -> p g k", k=K)
                    nc.vector.tensor_tensor(
                        neg3, comp(3),
                        cls[:, gcol].unsqueeze(2).broadcast_to((P, B, K)),
                        OP.not_equal)

                    mn = cmp.tile([P, B], F32, tag="mn")
                    nc.vector.tensor_reduce(
                        mn[:].unsqueeze(2), d23, AX.X, op=OP.min)
                    nc.vector.tensor_tensor(
                        d23, d23, mn[:].unsqueeze(2).broadcast_to((P, B, K)),
                        OP.subtract)
                    e = cmp.tile([P, SLOTS], F32, tag="e")
                    nc.scalar.activation(
                        e[:], d2[:], mybir.ActivationFunctionType.Exp,
                        scale=-float(CCBETA))
                    e3 = e[:].rearrange("p (g k) -> p g k", k=K)

                    den = cmp.tile([P, B], F32, tag="den")
                    nc.vector.tensor_reduce(
                        den[:].unsqueeze(2), e3, AX.X, op=OP.add)
                    nc.vector.tensor_tensor(e3, e3, neg3, OP.mult)
                    num = cmp.tile([P, B], F32, tag="num")
                    nc.vector.tensor_reduce(
                        num[:].unsqueeze(2), e3, AX.X, op=OP.add)
                    rec = cmp.tile([P, B], F32, tag="rec")
                    nc.vector.reciprocal(rec[:], den[:])
                    nc.vector.tensor_tensor(out_sb[:, gcol], num[:], rec[:],
                                            OP.mult)

            nc.sync.dma_start(out=out.ap(), in_=out_sb[:])

    nc.compile()
    return nc


_CACHE = {}


def _get_nc(S=FULL_S, G=FULL_G):
    key = (S, G)
    if key not in _CACHE:
        _CACHE[key] = build(S, G)
    return _CACHE[key]


def make_in_maps(p, labels, neighbor_idx, S=FULL_S, G=FULL_G):
    """Shard + lay out inputs. Point n of core c (n in [0, SP)) lives at
    partition n%128, slot n//128; host-permuted arrays put it at row
    (n%128)*G + n//128. The permuted global table row of real point j is
    c*SP + (r%128)*G + r//128 with c = j//S, r = j%S."""
    SP = P * G
    CALLS = G // B
    n_total = p.shape[0]
    assert n_total == N_CORES * S
    in_maps = []
    for c in range(N_CORES):
        rows = slice(c * S, (c + 1) * S)
        lab_c = np.zeros((SP, C), dtype=np.float32)
        p_c = np.zeros((SP, 3), dtype=np.float32)
        # permuted layout: row p*G + g  <- point g*128 + p (if real)
        pos = np.arange(SP)                  # pos = p*G + g
        pp, gg = pos // G, pos % G
        n_of_pos = gg * P + pp               # point id at this row
        valid = n_of_pos < S
        src = np.where(valid, n_of_pos, 0)
        lab_c[valid] = labels[rows][src[valid]]
        p_c[valid] = p[rows][src[valid]]

        # pair index arrays
        j = neighbor_idx[rows].astype(np.int64)        # [S, K] global ids
        jc, jr = j // S, j % S
        jperm = jc * SP + (jr % P).astype(np.int64) * G + jr // P
        jp = np.zeros((SP, K), dtype=np.int64)
        jp[:S] = jperm
        # arrange by (g, p, k): block[g][p][k] -> list i = (gs*K+k)*P + p
        blk = jp.reshape(G, P, K)
        q16_all = np.empty((CALLS, P, NW_CALL), dtype=np.int16)
        a_all = np.empty((CALLS, P, SLOTS), dtype=np.int8)
        for call in range(CALLS):
            sub = blk[call * B:(call + 1) * B]            # [B, P, K]
            lst = sub.transpose(0, 2, 1).reshape(SLOTS, P)  # [t, p] ; i=t*P+p
            q16 = (lst >> 4).astype(np.int16)
            a8 = (lst & 15).astype(np.int8)
            wrapped = q16.reshape(-1).reshape(NW_CALL, 16).T  # [16, NW_CALL]
            q16_all[call] = np.tile(wrapped, (8, 1))
            a_all[call] = a8.T                               # [p, t]
        in_maps.append({"labels": lab_c, "p3": p_c,
                        "qidx": q16_all, "aidx": a_all})
    return in_maps


def run(p, labels, neighbor_idx, S=FULL_S, G=FULL_G, trace=False):
    nc = _get_nc(S, G)
    in_maps = make_in_maps(p, labels, neighbor_idx, S, G)
    res = bass_utils.run_bass_kernel_spmd(
        nc, in_maps, core_ids=list(range(N_CORES)), trace=trace)
    outs = []
    for c in range(N_CORES):
        o2 = res.results[c]["out"].reshape(P, G)      # [p, g]
        outs.append(o2.T.reshape(-1)[:S])             # point n = g*128+p
    return np.concatenate(outs, axis=0).astype(np.float32), res


def kernel(p, labels, neighbor_idx):
    p = np.asarray(p, dtype=np.float32)
    labels = np.asarray(labels, dtype=np.float32)
    neighbor_idx = np.asarray(neighbor_idx)
    out, _ = run(p, labels, neighbor_idx)
    return out


# revision 11
# speedup vs baseline: 1.9660x; 1.9660x over previous
"""Trainium2 Bass kernel for the AmbiguityHead (retrieval_knn) problem.

Reference computation (per point i, K=15 neighbors j = nidx[i,k]):
    center_cls = argmax(labels[i])          (first occurrence on ties)
    neigh_cls  = argmax(labels[j])
    posmask    = center_cls == neigh_cls
    d2         = ||p[i] - p[j]||^2
    w          = softmax(-CCBETA * d2 over k)
    out[i]     = NU * sum_k w_k * (1 - posmask_k)

Strategy (8 NeuronCores, data-parallel over points; see spec sharding_hint):
  - Each core owns 62,500 points (padded to 63,488 = 128*496, laid out
    point-interleaved: point n -> partition n%128, slot n//128).
  - Phase 1: per-shard argmax class (encoded 13-argmax to keep jnp first-
    occurrence tie semantics), pack 16B records (x,y,z,clsenc), AllGather
    the packed table (8 x 63,488 rows) into pair-shared HBM.
  - Phase 2: the per-pair random gather uses the custom InstDMAGatherAnt
    ucode op: 256B super-rows (16 records) indexed by int16 j>>4 (the
    permuted table has 31,744 super-rows < 32768), issued over the 4 SWDGE
    queues. The 16B record is then extracted on-chip with a 4-level binary
    select tree keyed on bits of j&15, followed by the distance/softmax/
    compare reduction on the Vector/Scalar engines.
  - Host side does only sharding/layout: row permutations, index
    re-encoding (j -> [permuted row]>>4 / &15, int16 wrapped layout), and
    inverse permutation of the output. All arithmetic of the reference
    (argmax, distances, exp, reductions, compares) runs on device.
"""

import numpy as np

import concourse.bass as bass
import concourse.mybir as mybir
import concourse.tile as tile
from concourse import bacc
from concourse import bass_utils

F32 = mybir.dt.float32
I32 = mybir.dt.int32
I16 = mybir.dt.int16
I8 = mybir.dt.int8
AX = mybir.AxisListType
OP = mybir.AluOpType

N_CORES = 8
P = 128
C = 13
K = 15
CCBETA = 2.0
NU = 1.0

FULL_S = 500_000 // N_CORES   # real points per core
FULL_G = 496                  # point slots per partition (128*496 = 63488)
B = 4                         # point-groups (of 128) per gather call
NIDX_CALL = P * K * B         # 7680 indices per gather call
NW_CALL = NIDX_CALL // 16     # 480 idx per partition line (wrapped int16)
SLOTS = B * K                 # 60 pair slots per partition per call
DMA_SCRATCH = 65536
N_QUEUES = 4


def raw_dma_gather(nc, out_ap, in_ap, idxs_ap, num_idxs, elem_size, elem_step,
                   queue_num):
    """bass.dma_gather for elem_size*dtype % 256 != 0 (the ucode only needs
    the stride to be a 256B multiple for non-transpose HBM gathers; the
    bass-level assert is transpose-only in the ucode)."""
    eng = nc.gpsimd
    stride_bytes = elem_step * mybir.dt.size(in_ap.dtype)
    stride_bytes_256 = stride_bytes // 256
    assert stride_bytes % 256 == 0 and 0 < stride_bytes_256 < 256
    _in_ap = eng.lower_ap_dma(in_ap, for_custom_bir_dma=True)
    _idxs_ap = eng.lower_ap(idxs_ap)
    _out_ap = eng.lower_ap(out_ap)
    return eng.add_instruction(
        mybir.InstDMAGatherAnt(
            name=nc.get_next_instruction_name(),
            ins=[*_in_ap, _idxs_ap, eng.lower_val_access(eng.to_reg(num_idxs))],
            outs=[_out_ap],
            transpose=False,
            num_idxs=num_idxs,
            elem_size=elem_size,
            stride_bytes_256=stride_bytes_256,
            gen_mode=0,
            single_packet=False,
            queue_num=queue_num,
            sbuf_tokens_per_rank=0,
            sbuf_free_dim_per_rank=0,
            sbuf_free_dim_pad_per_rank=0,
            sbuf_byte_offset=0,
        ))


def select3(nc, out, mask, on_true, on_false):
    """nc.vector.select with un-merged APs so all three operands keep the
    same 3D shape (the stock path flattens only the contiguous out AP)."""
    v = nc.vector
    v.tensor_copy(out, on_false)
    return v.add_instruction(
        mybir.InstCopyPredicated(
            name=nc.get_next_instruction_name(),
            ins=[v.lower_ap(mask, opt=False), v.lower_ap(on_true, opt=False)],
            outs=[v.lower_ap(out, opt=False)],
        ))


def build(S=FULL_S, G=FULL_G):
    SP = P * G
    assert SP >= S and G % B == 0
    CALLS = G // B
    NTAB = N_CORES * SP          # permuted global table rows
    assert NTAB // 16 <= 32767   # int16 super-row indices

    nc = bacc.Bacc("TRN2", target_bir_lowering=False, debug=False,
                   num_devices=N_CORES, dynamic_dma_scratch_size=DMA_SCRATCH,
                   num_swdge_queues=N_QUEUES)

    # host-permuted inputs: row p*G+g holds point g*128+p of this core
    labels = nc.dram_tensor("labels", [SP, C], F32, kind="ExternalInput")
    p3 = nc.dram_tensor("p3", [SP, 3], F32, kind="ExternalInput")
    # per call: wrapped int16 super-row idxs [128, NW_CALL] and 16 one-hot
    # int8 mask planes (plane a marks slots whose record sits at sub-offset a)
    qidx = nc.dram_tensor("qidx", [CALLS, P, NW_CALL], I16, kind="ExternalInput")
    aidx = nc.dram_tensor("aidx", [CALLS, P, 16 * SLOTS], I8, kind="ExternalInput")
    out = nc.dram_tensor("out", [P, G], F32, kind="ExternalOutput")

    tshard = nc.dram_tensor("tshard", [SP, 4], F32)
    table_b = nc.dram_tensor("table_b", [NTAB, 4], F32, addr_space="Shared")
    tab64 = table_b.ap().rearrange("(q s) c -> q (s c)", s=16)  # [NTAB/16, 64]

    with tile.TileContext(nc) as tc:
        with tc.tile_pool(name="pers", bufs=1) as pers:
            # ---------------- phase 1: packed class/coord table ----------------
            with tc.tile_pool(name="ph1", bufs=1) as ph1:
                lab = ph1.tile([P, G * C], F32)
                nc.sync.dma_start(
                    out=lab[:], in_=labels.ap().rearrange("(p g) c -> p (g c)", p=P))
                lab3 = lab[:].rearrange("p (g c) -> p g c", c=C)

                m = ph1.tile([P, G], F32)
                nc.vector.tensor_reduce(m[:].unsqueeze(2), lab3, AX.X, op=OP.max)

                revio_i = ph1.tile([P, C], I32)
                nc.gpsimd.iota(revio_i[:], pattern=[[-1, C]], base=C,
                               channel_multiplier=0)
                revio = ph1.tile([P, C], F32)
                nc.vector.tensor_copy(revio[:], revio_i[:])

                eq = ph1.tile([P, G * C], F32)
                eq3 = eq[:].rearrange("p (g c) -> p g c", c=C)
                nc.vector.tensor_tensor(
                    eq3, lab3, m[:].unsqueeze(2).broadcast_to((P, G, C)),
                    OP.is_equal)
                nc.vector.tensor_tensor(
                    eq3, eq3, revio[:].unsqueeze(1).broadcast_to((P, G, C)),
                    OP.mult)
                cls = pers.tile([P, G], F32)
                nc.vector.tensor_reduce(cls[:].unsqueeze(2), eq3, AX.X, op=OP.max)

                pt = pers.tile([P, G * 3], F32)
                nc.sync.dma_start(
                    out=pt[:], in_=p3.ap().rearrange("(p g) c -> p (g c)", p=P))
                pt3 = pt[:].rearrange("p (g c) -> p g c", c=3)

                pk = ph1.tile([P, G * 4], F32)
                pk3 = pk[:].rearrange("p (g c) -> p g c", c=4)
                nc.vector.tensor_copy(pk3[:, :, 0:3], pt3)
                nc.vector.tensor_copy(pk3[:, :, 3:4], cls[:].unsqueeze(2))

                nc.sync.dma_start(
                    out=tshard.ap().rearrange("(p g) c -> p (g c)", p=P),
                    in_=pk[:])

                nc.gpsimd.collective_compute(
                    "AllGather", OP.bypass,
                    replica_groups=[list(range(N_CORES))],
                    ins=[tshard.ap().opt()],
                    outs=[table_b.ap().opt()],
                )

            # ---------------- phase 2: gather + extract + reduce --------------
            out_sb = pers.tile([P, G], F32)
            with tc.tile_pool(name="io", bufs=4) as iop, \
                 tc.tile_pool(name="cmp", bufs=2) as cmp:
                for call in range(CALLS):
                    gcol = slice(call * B, (call + 1) * B)
                    qt = iop.tile([P, NW_CALL], I16, tag="qt")
                    nc.sync.dma_start(out=qt[:], in_=qidx[call, :, :])
                    at = iop.tile([P, 16 * SLOTS], I8, tag="at")
                    nc.sync.dma_start(out=at[:], in_=aidx[call, :, :])
                    at3 = at[:].rearrange("p (a n) -> p a n", a=16)

                    gat = iop.tile([P, SLOTS * 64], F32, tag="gat")
                    nc.gpsimd.dma_gather(
                        out_ap=gat[:].rearrange("p (n e) -> p n e", e=64),
                        in_ap=tab64,
                        idxs_ap=qt[:],
                        num_idxs=NIDX_CALL,
                        num_idxs_reg=NIDX_CALL,
                        elem_size=64,
                        single_packet=False,
                        queue_num=call % N_QUEUES,
                    )
                    g3 = gat[:].rearrange("p (n e) -> p n e", e=64)

                    # one-hot extraction: plane a overwrites the slots
                    # whose 16B record sits at sub-offset a of the super-row
                    ext = cmp.tile([P, SLOTS * 4], F32, tag="ext")
                    extv = ext[:].rearrange("p (n e) -> p n e", e=4)
                    for a in range(16):
                        nc.vector.add_instruction(
                            mybir.InstCopyPredicated(
                                name=nc.get_next_instruction_name(),
                                ins=[
                                    nc.vector.lower_ap(
                                        at3[:, a, :].unsqueeze(2)
                                        .broadcast_to((P, SLOTS, 4)), opt=False),
                                    nc.vector.lower_ap(
                                        g3[:, :, 4 * a:4 * a + 4], opt=False),
                                ],
                                outs=[nc.vector.lower_ap(extv, opt=False)],
                            ))

                    # slot n = g_sub*15 + k ; component views [P, B, K]
                    def comp(cidx):
                        return ext[:].rearrange(
                            "p (g k e) -> p g k e", k=K, e=4)[:, :, :, cidx:cidx + 1
                                                             ].squeeze(3)

                    def cb(cidx):
                        return pt3[:, gcol, cidx:cidx + 1].broadcast_to((P, B, K))

                    dx = cmp.tile([P, SLOTS], F32, tag="dx")
                    dx3 = dx[:].rearrange("p (g k) -> p g k", k=K)
                    dy = cmp.tile([P, SLOTS], F32, tag="dy")
                    dy3 = dy[:].rearrange("p (g k) -> p g k", k=K)
                    dz = cmp.tile([P, SLOTS], F32, tag="dz")
                    dz3 = dz[:].rearrange("p (g k) -> p g k", k=K)
                    d2 = cmp.tile([P, SLOTS], F32, tag="d2")
                    d23 = d2[:].rearrange("p (g k) -> p g k", k=K)

                    nc.vector.tensor_tensor(dx3, comp(0), cb(0), OP.subtract)
                    nc.vector.tensor_tensor(dy3, comp(1), cb(1), OP.subtract)
                    nc.vector.tensor_tensor(dz3, comp(2), cb(2), OP.subtract)
                    nc.vector.tensor_tensor(d23, dx3, dx3, OP.mult)
                    nc.vector.tensor_tensor(dy3, dy3, dy3, OP.mult)
                    nc.vector.tensor_tensor(d23, d23, dy3, OP.add)
                    nc.vector.tensor_tensor(dz3, dz3, dz3, OP.mult)
                    nc.vector.tensor_tensor(d23, d23, dz3, OP.add)

                    neg = cmp.tile([P, SLOTS], F32, tag="neg")
                    neg3 = neg[:].rearrange("p (g k) -> p g k", k=K)
                    nc.vector.tensor_tensor(
                        neg3, comp(3),
                        cls[:, gcol].unsqueeze(2).broadcast_to((P, B, K)),
                        OP.not_equal)

                    mn = cmp.tile([P, B], F32, tag="mn")
                    nc.vector.tensor_reduce(
                        mn[:].unsqueeze(2), d23, AX.X, op=OP.min)
                    nc.vector.tensor_tensor(
                        d23, d23, mn[:].unsqueeze(2).broadcast_to((P, B, K)),
                        OP.subtract)
                    e = cmp.tile([P, SLOTS], F32, tag="e")
                    nc.scalar.activation(
                        e[:], d2[:], mybir.ActivationFunctionType.Exp,
                        scale=-float(CCBETA))
                    e3 = e[:].rearrange("p (g k) -> p g k", k=K)

                    den = cmp.tile([P, B], F32, tag="den")
                    nc.vector.tensor_reduce(
                        den[:].unsqueeze(2), e3, AX.X, op=OP.add)
                    nc.vector.tensor_tensor(e3, e3, neg3, OP.mult)
                    num = cmp.tile([P, B], F32, tag="num")
                    nc.vector.tensor_reduce(
                        num[:].unsqueeze(2), e3, AX.X, op=OP.add)
                    rec = cmp.tile([P, B], F32, tag="rec")
                    nc.vector.reciprocal(rec[:], den[:])
                    nc.vector.tensor_tensor(out_sb[:, gcol], num[:], rec[:],
                                            OP.mult)

            nc.sync.dma_start(out=out.ap(), in_=out_sb[:])

    nc.compile()
    return nc


_CACHE = {}


def _get_nc(S=FULL_S, G=FULL_G):
    key = (S, G)
    if key not in _CACHE:
        _CACHE[key] = build(S, G)
    return _CACHE[key]


def make_in_maps(p, labels, neighbor_idx, S=FULL_S, G=FULL_G):
    """Shard + lay out inputs. Point n of core c (n in [0, SP)) lives at
    partition n%128, slot n//128; host-permuted arrays put it at row
    (n%128)*G + n//128. The permuted global table row of real point j is
    c*SP + (r%128)*G + r//128 with c = j//S, r = j%S."""
    SP = P * G
    CALLS = G // B
    n_total = p.shape[0]
    assert n_total == N_CORES * S
    in_maps = []
    for c in range(N_CORES):
        rows = slice(c * S, (c + 1) * S)
        lab_c = np.zeros((SP, C), dtype=np.float32)
        p_c = np.zeros((SP, 3), dtype=np.float32)
        # permuted layout: row p*G + g  <- point g*128 + p (if real)
        pos = np.arange(SP)                  # pos = p*G + g
        pp, gg = pos // G, pos % G
        n_of_pos = gg * P + pp               # point id at this row
        valid = n_of_pos < S
        src = np.where(valid, n_of_pos, 0)
        lab_c[valid] = labels[rows][src[valid]]
        p_c[valid] = p[rows][src[valid]]

        # pair index arrays
        j = neighbor_idx[rows].astype(np.int64)        # [S, K] global ids
        jc, jr = j // S, j % S
        jperm = jc * SP + (jr % P).astype(np.int64) * G + jr // P
        jp = np.zeros((SP, K), dtype=np.int64)
        jp[:S] = jperm
        # arrange by (g, p, k): block[g][p][k] -> list i = (gs*K+k)*P + p
        blk = jp.reshape(G, P, K)
        q16_all = np.empty((CALLS, P, NW_CALL), dtype=np.int16)
        a_all = np.empty((CALLS, P, 16 * SLOTS), dtype=np.int8)
        plane = np.arange(16).reshape(16, 1, 1)
        for call in range(CALLS):
            sub = blk[call * B:(call + 1) * B]            # [B, P, K]
            lst = sub.transpose(0, 2, 1).reshape(SLOTS, P)  # [t, p] ; i=t*P+p
            q16 = (lst >> 4).astype(np.int16)
            a8 = (lst & 15).astype(np.int8)
            wrapped = q16.reshape(-1).reshape(NW_CALL, 16).T  # [16, NW_CALL]
            q16_all[call] = np.tile(wrapped, (8, 1))
            onehot = (a8.T[None, :, :] == plane)             # [16, p, t]
            a_all[call] = onehot.transpose(1, 0, 2).reshape(
                P, 16 * SLOTS).astype(np.int8)
        in_maps.append({"labels": lab_c, "p3": p_c,
                        "qidx": q16_all, "aidx": a_all})
    return in_maps


def run(p, labels, neighbor_idx, S=FULL_S, G=FULL_G, trace=False):
    nc = _get_nc(S, G)
    in_maps = make_in_maps(p, labels, neighbor_idx, S, G)
    res = bass_utils.run_bass_kernel_spmd(
        nc, in_maps, core_ids=list(range(N_CORES)), trace=trace)
    outs = []
    for c in range(N_CORES):
        o2 = res.results[c]["out"].reshape(P, G)      # [p, g]
        outs.append(o2.T.reshape(-1)[:S])             # point n = g*128+p
    return np.concatenate(outs, axis=0).astype(np.float32), res


def kernel(p, labels, neighbor_idx):
    p = np.asarray(p, dtype=np.float32)
    labels = np.asarray(labels, dtype=np.float32)
    neighbor_idx = np.asarray(neighbor_idx)
    out, _ = run(p, labels, neighbor_idx)
    return out


# revision 12
# speedup vs baseline: 2.7776x; 1.4129x over previous
"""Trainium2 Bass kernel for the AmbiguityHead (retrieval_knn) problem.

Reference computation (per point i, K=15 neighbors j = nidx[i,k]):
    center_cls = argmax(labels[i])          (first occurrence on ties)
    neigh_cls  = argmax(labels[j])
    posmask    = center_cls == neigh_cls
    d2         = ||p[i] - p[j]||^2
    w          = softmax(-CCBETA * d2 over k)
    out[i]     = NU * sum_k w_k * (1 - posmask_k)

Strategy (8 NeuronCores, data-parallel over points; see spec sharding_hint):
  - Each core owns 62,500 points (padded to 63,488 = 128*496, laid out
    point-interleaved: point n -> partition n%128, slot n//128).
  - Phase 1: per-shard argmax class (encoded 13-argmax to keep jnp first-
    occurrence tie semantics), pack 16B records (x,y,z,clsenc), AllGather
    the packed table (8 x 63,488 rows) into pair-shared HBM.
  - Phase 2: the per-pair random gather uses the custom InstDMAGatherAnt
    ucode op: 256B super-rows (16 records) indexed by int16 j>>4 (the
    permuted table has 31,744 super-rows < 32768), issued over the 4 SWDGE
    queues. The 16B record is then extracted on-chip with a 4-level binary
    select tree keyed on bits of j&15, followed by the distance/softmax/
    compare reduction on the Vector/Scalar engines.
  - Host side does only sharding/layout: row permutations, index
    re-encoding (j -> [permuted row]>>4 / &15, int16 wrapped layout), and
    inverse permutation of the output. All arithmetic of the reference
    (argmax, distances, exp, reductions, compares) runs on device.
"""

import numpy as np

import concourse.bass as bass
import concourse.mybir as mybir
import concourse.tile as tile
from concourse import bacc
from concourse import bass_utils

F32 = mybir.dt.float32
I32 = mybir.dt.int32
I16 = mybir.dt.int16
I8 = mybir.dt.int8
AX = mybir.AxisListType
OP = mybir.AluOpType

N_CORES = 8
P = 128
C = 13
K = 15
CCBETA = 2.0
NU = 1.0

FULL_S = 500_000 // N_CORES   # real points per core
FULL_G = 496                  # point slots per partition (128*496 = 63488)
B = 4                         # point-groups (of 128) per gather call
NIDX_CALL = P * K * B         # 7680 indices per gather call
NW_CALL = NIDX_CALL // 16     # 480 idx per partition line (wrapped int16)
SLOTS = B * K                 # 60 pair slots per partition per call
DMA_SCRATCH = 65536
N_QUEUES = 4


def raw_dma_gather(nc, out_ap, in_ap, idxs_ap, num_idxs, elem_size, elem_step,
                   queue_num):
    """bass.dma_gather for elem_size*dtype % 256 != 0 (the ucode only needs
    the stride to be a 256B multiple for non-transpose HBM gathers; the
    bass-level assert is transpose-only in the ucode)."""
    eng = nc.gpsimd
    stride_bytes = elem_step * mybir.dt.size(in_ap.dtype)
    stride_bytes_256 = stride_bytes // 256
    assert stride_bytes % 256 == 0 and 0 < stride_bytes_256 < 256
    _in_ap = eng.lower_ap_dma(in_ap, for_custom_bir_dma=True)
    _idxs_ap = eng.lower_ap(idxs_ap)
    _out_ap = eng.lower_ap(out_ap)
    return eng.add_instruction(
        mybir.InstDMAGatherAnt(
            name=nc.get_next_instruction_name(),
            ins=[*_in_ap, _idxs_ap, eng.lower_val_access(eng.to_reg(num_idxs))],
            outs=[_out_ap],
            transpose=False,
            num_idxs=num_idxs,
            elem_size=elem_size,
            stride_bytes_256=stride_bytes_256,
            gen_mode=0,
            single_packet=False,
            queue_num=queue_num,
            sbuf_tokens_per_rank=0,
            sbuf_free_dim_per_rank=0,
            sbuf_free_dim_pad_per_rank=0,
            sbuf_byte_offset=0,
        ))


def select3(nc, out, mask, on_true, on_false):
    """nc.vector.select with un-merged APs so all three operands keep the
    same 3D shape (the stock path flattens only the contiguous out AP)."""
    v = nc.vector
    v.tensor_copy(out, on_false)
    return v.add_instruction(
        mybir.InstCopyPredicated(
            name=nc.get_next_instruction_name(),
            ins=[v.lower_ap(mask, opt=False), v.lower_ap(on_true, opt=False)],
            outs=[v.lower_ap(out, opt=False)],
        ))


def build(S=FULL_S, G=FULL_G):
    SP = P * G
    assert SP >= S and G % B == 0
    CALLS = G // B
    NTAB = N_CORES * SP          # permuted global table rows
    assert NTAB // 16 <= 32767   # int16 super-row indices

    nc = bacc.Bacc("TRN2", target_bir_lowering=False, debug=False,
                   num_devices=N_CORES, dynamic_dma_scratch_size=DMA_SCRATCH,
                   num_swdge_queues=N_QUEUES)

    # host-permuted inputs: row p*G+g holds point g*128+p of this core
    labels = nc.dram_tensor("labels", [SP, C], F32, kind="ExternalInput")
    p3 = nc.dram_tensor("p3", [SP, 3], F32, kind="ExternalInput")
    # per call: wrapped int16 super-row idxs [128, NW_CALL] and 16 one-hot
    # int8 mask planes (plane a marks slots whose record sits at sub-offset a)
    qidx = nc.dram_tensor("qidx", [CALLS, P, NW_CALL], I16, kind="ExternalInput")
    aidx = nc.dram_tensor("aidx", [CALLS, P, 16 * SLOTS], I8, kind="ExternalInput")
    out = nc.dram_tensor("out", [P, G], F32, kind="ExternalOutput")

    tshard = nc.dram_tensor("tshard", [SP, 4], F32)
    table_b = nc.dram_tensor("table_b", [NTAB, 4], F32, addr_space="Shared")
    tab64 = table_b.ap().rearrange("(q s) c -> q (s c)", s=16)  # [NTAB/16, 64]

    with tile.TileContext(nc) as tc:
        with tc.tile_pool(name="pers", bufs=1) as pers:
            # ---------------- phase 1: packed class/coord table ----------------
            with tc.tile_pool(name="ph1", bufs=1) as ph1:
                lab = ph1.tile([P, G * C], F32)
                nc.sync.dma_start(
                    out=lab[:], in_=labels.ap().rearrange("(p g) c -> p (g c)", p=P))
                lab3 = lab[:].rearrange("p (g c) -> p g c", c=C)

                m = ph1.tile([P, G], F32)
                nc.vector.tensor_reduce(m[:].unsqueeze(2), lab3, AX.X, op=OP.max)

                revio_i = ph1.tile([P, C], I32)
                nc.gpsimd.iota(revio_i[:], pattern=[[-1, C]], base=C,
                               channel_multiplier=0)
                revio = ph1.tile([P, C], F32)
                nc.vector.tensor_copy(revio[:], revio_i[:])

                eq = ph1.tile([P, G * C], F32)
                eq3 = eq[:].rearrange("p (g c) -> p g c", c=C)
                nc.vector.tensor_tensor(
                    eq3, lab3, m[:].unsqueeze(2).broadcast_to((P, G, C)),
                    OP.is_equal)
                nc.vector.tensor_tensor(
                    eq3, eq3, revio[:].unsqueeze(1).broadcast_to((P, G, C)),
                    OP.mult)
                cls = pers.tile([P, G], F32)
                nc.vector.tensor_reduce(cls[:].unsqueeze(2), eq3, AX.X, op=OP.max)

                pt = pers.tile([P, G * 3], F32)
                nc.sync.dma_start(
                    out=pt[:], in_=p3.ap().rearrange("(p g) c -> p (g c)", p=P))
                pt3 = pt[:].rearrange("p (g c) -> p g c", c=3)

                pk = ph1.tile([P, G * 4], F32)
                pk3 = pk[:].rearrange("p (g c) -> p g c", c=4)
                nc.vector.tensor_copy(pk3[:, :, 0:3], pt3)
                nc.vector.tensor_copy(pk3[:, :, 3:4], cls[:].unsqueeze(2))

                nc.sync.dma_start(
                    out=tshard.ap().rearrange("(p g) c -> p (g c)", p=P),
                    in_=pk[:])

                nc.gpsimd.collective_compute(
                    "AllGather", OP.bypass,
                    replica_groups=[list(range(N_CORES))],
                    ins=[tshard.ap().opt()],
                    outs=[table_b.ap().opt()],
                )

            # ---------------- phase 2: gather + extract + reduce --------------
            out_sb = pers.tile([P, G], F32)
            with tc.tile_pool(name="io", bufs=5) as iop, \
                 tc.tile_pool(name="cmp", bufs=2) as cmp:
                for call in range(CALLS):
                    gcol = slice(call * B, (call + 1) * B)
                    qt = iop.tile([P, NW_CALL], I16, tag="qt")
                    nc.sync.dma_start(out=qt[:], in_=qidx[call, :, :])
                    at = iop.tile([P, 16 * SLOTS], I8, tag="at")
                    nc.sync.dma_start(out=at[:], in_=aidx[call, :, :])
                    at3 = at[:].rearrange("p (a n) -> p a n", a=16)

                    gat = iop.tile([P, SLOTS * 64], F32, tag="gat")
                    nc.gpsimd.dma_gather(
                        out_ap=gat[:].rearrange("p (n e) -> p n e", e=64),
                        in_ap=tab64,
                        idxs_ap=qt[:],
                        num_idxs=NIDX_CALL,
                        num_idxs_reg=NIDX_CALL,
                        elem_size=64,
                        single_packet=False,
                        queue_num=call % N_QUEUES,
                    )
                    g3 = gat[:].rearrange("p (n e) -> p n e", e=64)

                    # one-hot extraction: plane a overwrites the slots
                    # whose 16B record sits at sub-offset a of the super-row
                    ext = cmp.tile([P, SLOTS * 4], F32, tag="ext")
                    extv = ext[:].rearrange("p (n e) -> p n e", e=4)
                    for a in range(16):
                        nc.vector.add_instruction(
                            mybir.InstCopyPredicated(
                                name=nc.get_next_instruction_name(),
                                ins=[
                                    nc.vector.lower_ap(
                                        at3[:, a, :].unsqueeze(2)
                                        .broadcast_to((P, SLOTS, 4)), opt=False),
                                    nc.vector.lower_ap(
                                        g3[:, :, 4 * a:4 * a + 4], opt=False),
                                ],
                                outs=[nc.vector.lower_ap(extv, opt=False)],
                            ))

                    # slot n = g_sub*15 + k ; component views [P, B, K]
                    def comp(cidx):
                        return ext[:].rearrange(
                            "p (g k e) -> p g k e", k=K, e=4)[:, :, :, cidx:cidx + 1
                                                             ].squeeze(3)

                    def cb(cidx):
                        return pt3[:, gcol, cidx:cidx + 1].broadcast_to((P, B, K))

                    dx = cmp.tile([P, SLOTS], F32, tag="dx")
                    dx3 = dx[:].rearrange("p (g k) -> p g k", k=K)
                    dy = cmp.tile([P, SLOTS], F32, tag="dy")
                    dy3 = dy[:].rearrange("p (g k) -> p g k", k=K)
                    dz = cmp.tile([P, SLOTS], F32, tag="dz")
                    dz3 = dz[:].rearrange("p (g k) -> p g k", k=K)
                    d2 = cmp.tile([P, SLOTS], F32, tag="d2")
                    d23 = d2[:].rearrange("p (g k) -> p g k", k=K)

                    nc.vector.tensor_tensor(dx3, comp(0), cb(0), OP.subtract)
                    nc.vector.tensor_tensor(dy3, comp(1), cb(1), OP.subtract)
                    nc.vector.tensor_tensor(dz3, comp(2), cb(2), OP.subtract)
                    nc.vector.tensor_tensor(d23, dx3, dx3, OP.mult)
                    nc.vector.tensor_tensor(dy3, dy3, dy3, OP.mult)
                    nc.vector.tensor_tensor(d23, d23, dy3, OP.add)
                    nc.vector.tensor_tensor(dz3, dz3, dz3, OP.mult)
                    nc.vector.tensor_tensor(d23, d23, dz3, OP.add)

                    neg = cmp.tile([P, SLOTS], F32, tag="neg")
                    neg3 = neg[:].rearrange("p (g k) -> p g k", k=K)
                    nc.vector.tensor_tensor(
                        neg3, comp(3),
                        cls[:, gcol].unsqueeze(2).broadcast_to((P, B, K)),
                        OP.not_equal)

                    mn = cmp.tile([P, B], F32, tag="mn")
                    nc.vector.tensor_reduce(
                        mn[:].unsqueeze(2), d23, AX.X, op=OP.min)
                    nc.vector.tensor_tensor(
                        d23, d23, mn[:].unsqueeze(2).broadcast_to((P, B, K)),
                        OP.subtract)
                    e = cmp.tile([P, SLOTS], F32, tag="e")
                    nc.scalar.activation(
                        e[:], d2[:], mybir.ActivationFunctionType.Exp,
                        scale=-float(CCBETA))
                    e3 = e[:].rearrange("p (g k) -> p g k", k=K)

                    den = cmp.tile([P, B], F32, tag="den")
                    nc.vector.tensor_reduce(
                        den[:].unsqueeze(2), e3, AX.X, op=OP.add)
                    nc.vector.tensor_tensor(e3, e3, neg3, OP.mult)
                    num = cmp.tile([P, B], F32, tag="num")
                    nc.vector.tensor_reduce(
                        num[:].unsqueeze(2), e3, AX.X, op=OP.add)
                    rec = cmp.tile([P, B], F32, tag="rec")
                    nc.vector.reciprocal(rec[:], den[:])
                    nc.vector.tensor_tensor(out_sb[:, gcol], num[:], rec[:],
                                            OP.mult)

            nc.sync.dma_start(out=out.ap(), in_=out_sb[:])

    nc.compile()
    return nc


_CACHE = {}


def _get_nc(S=FULL_S, G=FULL_G):
    key = (S, G)
    if key not in _CACHE:
        _CACHE[key] = build(S, G)
    return _CACHE[key]


def make_in_maps(p, labels, neighbor_idx, S=FULL_S, G=FULL_G):
    """Shard + lay out inputs. Point n of core c (n in [0, SP)) lives at
    partition n%128, slot n//128; host-permuted arrays put it at row
    (n%128)*G + n//128. The permuted global table row of real point j is
    c*SP + (r%128)*G + r//128 with c = j//S, r = j%S."""
    SP = P * G
    CALLS = G // B
    n_total = p.shape[0]
    assert n_total == N_CORES * S
    in_maps = []
    for c in range(N_CORES):
        rows = slice(c * S, (c + 1) * S)
        lab_c = np.zeros((SP, C), dtype=np.float32)
        p_c = np.zeros((SP, 3), dtype=np.float32)
        # permuted layout: row p*G + g  <- point g*128 + p (if real)
        pos = np.arange(SP)                  # pos = p*G + g
        pp, gg = pos // G, pos % G
        n_of_pos = gg * P + pp               # point id at this row
        valid = n_of_pos < S
        src = np.where(valid, n_of_pos, 0)
        lab_c[valid] = labels[rows][src[valid]]
        p_c[valid] = p[rows][src[valid]]

        # pair index arrays
        j = neighbor_idx[rows].astype(np.int64)        # [S, K] global ids
        jc, jr = j // S, j % S
        jperm = jc * SP + (jr % P).astype(np.int64) * G + jr // P
        jp = np.zeros((SP, K), dtype=np.int64)
        jp[:S] = jperm
        # arrange by (g, p, k): block[g][p][k] -> list i = (gs*K+k)*P + p
        blk = jp.reshape(G, P, K)
        q16_all = np.empty((CALLS, P, NW_CALL), dtype=np.int16)
        a_all = np.empty((CALLS, P, 16 * SLOTS), dtype=np.int8)
        plane = np.arange(16).reshape(16, 1, 1)
        for call in range(CALLS):
            sub = blk[call * B:(call + 1) * B]            # [B, P, K]
            lst = sub.transpose(0, 2, 1).reshape(SLOTS, P)  # [t, p] ; i=t*P+p
            q16 = (lst >> 4).astype(np.int16)
            a8 = (lst & 15).astype(np.int8)
            wrapped = q16.reshape(-1).reshape(NW_CALL, 16).T  # [16, NW_CALL]
            q16_all[call] = np.tile(wrapped, (8, 1))
            onehot = (a8.T[None, :, :] == plane)             # [16, p, t]
            a_all[call] = onehot.transpose(1, 0, 2).reshape(
                P, 16 * SLOTS).astype(np.int8)
        in_maps.append({"labels": lab_c, "p3": p_c,
                        "qidx": q16_all, "aidx": a_all})
    return in_maps


def run(p, labels, neighbor_idx, S=FULL_S, G=FULL_G, trace=False):
    nc = _get_nc(S, G)
    in_maps = make_in_maps(p, labels, neighbor_idx, S, G)
    res = bass_utils.run_bass_kernel_spmd(
        nc, in_maps, core_ids=list(range(N_CORES)), trace=trace)
    outs = []
    for c in range(N_CORES):
        o2 = res.results[c]["out"].reshape(P, G)      # [p, g]
        outs.append(o2.T.reshape(-1)[:S])             # point n = g*128+p
    return np.concatenate(outs, axis=0).astype(np.float32), res


def kernel(p, labels, neighbor_idx):
    p = np.asarray(p, dtype=np.float32)
    labels = np.asarray(labels, dtype=np.float32)
    neighbor_idx = np.asarray(neighbor_idx)
    out, _ = run(p, labels, neighbor_idx)
    return out


# revision 13
# speedup vs baseline: 2.8139x; 1.0131x over previous
"""Trainium2 Bass kernel for the AmbiguityHead (retrieval_knn) problem.

Reference computation (per point i, K=15 neighbors j = nidx[i,k]):
    center_cls = argmax(labels[i])          (first occurrence on ties)
    neigh_cls  = argmax(labels[j])
    posmask    = center_cls == neigh_cls
    d2         = ||p[i] - p[j]||^2
    w          = softmax(-CCBETA * d2 over k)
    out[i]     = NU * sum_k w_k * (1 - posmask_k)

Strategy (8 NeuronCores, data-parallel over points; see spec sharding_hint):
  - Each core owns 62,500 points (padded to 63,488 = 128*496, laid out
    point-interleaved: point n -> partition n%128, slot n//128).
  - Phase 1: per-shard argmax class (encoded 13-argmax to keep jnp first-
    occurrence tie semantics), pack 16B records (x,y,z,clsenc), AllGather
    the packed table (8 x 63,488 rows) into pair-shared HBM.
  - Phase 2: the per-pair random gather uses the custom InstDMAGatherAnt
    ucode op: 256B super-rows (16 records) indexed by int16 j>>4 (the
    permuted table has 31,744 super-rows < 32768), issued over the 4 SWDGE
    queues. The 16B record is then extracted on-chip with a 4-level binary
    select tree keyed on bits of j&15, followed by the distance/softmax/
    compare reduction on the Vector/Scalar engines.
  - Host side does only sharding/layout: row permutations, index
    re-encoding (j -> [permuted row]>>4 / &15, int16 wrapped layout), and
    inverse permutation of the output. All arithmetic of the reference
    (argmax, distances, exp, reductions, compares) runs on device.
"""

import numpy as np

import concourse.bass as bass
import concourse.mybir as mybir
import concourse.tile as tile
from concourse import bacc
from concourse import bass_utils

F32 = mybir.dt.float32
I32 = mybir.dt.int32
I16 = mybir.dt.int16
I8 = mybir.dt.int8
AX = mybir.AxisListType
OP = mybir.AluOpType

N_CORES = 8
P = 128
C = 13
K = 15
CCBETA = 2.0
NU = 1.0

FULL_S = 500_000 // N_CORES   # real points per core
FULL_G = 496                  # point slots per partition (128*496 = 63488)
B = 4                         # point-groups (of 128) per gather call
NIDX_CALL = P * K * B         # 7680 indices per gather call
NW_CALL = NIDX_CALL // 16     # 480 idx per partition line (wrapped int16)
SLOTS = B * K                 # 60 pair slots per partition per call
DMA_SCRATCH = 65536
N_QUEUES = 4


def raw_dma_gather(nc, out_ap, in_ap, idxs_ap, num_idxs, elem_size, elem_step,
                   queue_num):
    """bass.dma_gather for elem_size*dtype % 256 != 0 (the ucode only needs
    the stride to be a 256B multiple for non-transpose HBM gathers; the
    bass-level assert is transpose-only in the ucode)."""
    eng = nc.gpsimd
    stride_bytes = elem_step * mybir.dt.size(in_ap.dtype)
    stride_bytes_256 = stride_bytes // 256
    assert stride_bytes % 256 == 0 and 0 < stride_bytes_256 < 256
    _in_ap = eng.lower_ap_dma(in_ap, for_custom_bir_dma=True)
    _idxs_ap = eng.lower_ap(idxs_ap)
    _out_ap = eng.lower_ap(out_ap)
    return eng.add_instruction(
        mybir.InstDMAGatherAnt(
            name=nc.get_next_instruction_name(),
            ins=[*_in_ap, _idxs_ap, eng.lower_val_access(eng.to_reg(num_idxs))],
            outs=[_out_ap],
            transpose=False,
            num_idxs=num_idxs,
            elem_size=elem_size,
            stride_bytes_256=stride_bytes_256,
            gen_mode=0,
            single_packet=False,
            queue_num=queue_num,
            sbuf_tokens_per_rank=0,
            sbuf_free_dim_per_rank=0,
            sbuf_free_dim_pad_per_rank=0,
            sbuf_byte_offset=0,
        ))


def select3(nc, out, mask, on_true, on_false):
    """nc.vector.select with un-merged APs so all three operands keep the
    same 3D shape (the stock path flattens only the contiguous out AP)."""
    v = nc.vector
    v.tensor_copy(out, on_false)
    return v.add_instruction(
        mybir.InstCopyPredicated(
            name=nc.get_next_instruction_name(),
            ins=[v.lower_ap(mask, opt=False), v.lower_ap(on_true, opt=False)],
            outs=[v.lower_ap(out, opt=False)],
        ))


def build(S=FULL_S, G=FULL_G):
    SP = P * G
    assert SP >= S and G % B == 0
    CALLS = G // B
    NTAB = N_CORES * SP          # permuted global table rows
    assert NTAB // 16 <= 32767   # int16 super-row indices

    nc = bacc.Bacc("TRN2", target_bir_lowering=False, debug=False,
                   num_devices=N_CORES, dynamic_dma_scratch_size=DMA_SCRATCH,
                   num_swdge_queues=N_QUEUES)

    # host-permuted inputs: row p*G+g holds point g*128+p of this core
    labels = nc.dram_tensor("labels", [SP, C], F32, kind="ExternalInput")
    p3 = nc.dram_tensor("p3", [SP, 3], F32, kind="ExternalInput")
    # per call: wrapped int16 super-row idxs [128, NW_CALL] and 16 one-hot
    # int8 mask planes (plane a marks slots whose record sits at sub-offset a)
    qidx = nc.dram_tensor("qidx", [CALLS, P, NW_CALL], I16, kind="ExternalInput")
    aidx = nc.dram_tensor("aidx", [CALLS, P, 16 * SLOTS], I8, kind="ExternalInput")
    out = nc.dram_tensor("out", [P, G], F32, kind="ExternalOutput")

    tshard = nc.dram_tensor("tshard", [SP, 4], F32)
    table_b = nc.dram_tensor("table_b", [NTAB, 4], F32, addr_space="Shared")
    tab64 = table_b.ap().rearrange("(q s) c -> q (s c)", s=16)  # [NTAB/16, 64]

    with tile.TileContext(nc) as tc:
        with tc.tile_pool(name="pers", bufs=1) as pers:
            # ---------------- phase 1: packed class/coord table ----------------
            with tc.tile_pool(name="ph1", bufs=1) as ph1:
                lab = ph1.tile([P, G * C], F32)
                nc.sync.dma_start(
                    out=lab[:], in_=labels.ap().rearrange("(p g) c -> p (g c)", p=P))
                lab3 = lab[:].rearrange("p (g c) -> p g c", c=C)

                m = ph1.tile([P, G], F32)
                nc.vector.tensor_reduce(m[:].unsqueeze(2), lab3, AX.X, op=OP.max)

                revio_i = ph1.tile([P, C], I32)
                nc.gpsimd.iota(revio_i[:], pattern=[[-1, C]], base=C,
                               channel_multiplier=0)
                revio = ph1.tile([P, C], F32)
                nc.vector.tensor_copy(revio[:], revio_i[:])

                eq = ph1.tile([P, G * C], F32)
                eq3 = eq[:].rearrange("p (g c) -> p g c", c=C)
                nc.vector.tensor_tensor(
                    eq3, lab3, m[:].unsqueeze(2).broadcast_to((P, G, C)),
                    OP.is_equal)
                nc.vector.tensor_tensor(
                    eq3, eq3, revio[:].unsqueeze(1).broadcast_to((P, G, C)),
                    OP.mult)
                cls = pers.tile([P, G], F32)
                nc.vector.tensor_reduce(cls[:].unsqueeze(2), eq3, AX.X, op=OP.max)

                pt = pers.tile([P, G * 3], F32)
                nc.sync.dma_start(
                    out=pt[:], in_=p3.ap().rearrange("(p g) c -> p (g c)", p=P))
                pt3 = pt[:].rearrange("p (g c) -> p g c", c=3)

                pk = ph1.tile([P, G * 4], F32)
                pk3 = pk[:].rearrange("p (g c) -> p g c", c=4)
                nc.vector.tensor_copy(pk3[:, :, 0:3], pt3)
                nc.vector.tensor_copy(pk3[:, :, 3:4], cls[:].unsqueeze(2))

                nc.sync.dma_start(
                    out=tshard.ap().rearrange("(p g) c -> p (g c)", p=P),
                    in_=pk[:])

                nc.gpsimd.collective_compute(
                    "AllGather", OP.bypass,
                    replica_groups=[list(range(N_CORES))],
                    ins=[tshard.ap().opt()],
                    outs=[table_b.ap().opt()],
                )

            # ---------------- phase 2: gather + extract + reduce --------------
            out_sb = pers.tile([P, G], F32)
            with tc.tile_pool(name="io", bufs=6) as iop, \
                 tc.tile_pool(name="cmp", bufs=2) as cmp:
                for call in range(CALLS):
                    gcol = slice(call * B, (call + 1) * B)
                    qt = iop.tile([P, NW_CALL], I16, tag="qt")
                    nc.sync.dma_start(out=qt[:], in_=qidx[call, :, :])
                    at = iop.tile([P, 16 * SLOTS], I8, tag="at")
                    nc.sync.dma_start(out=at[:], in_=aidx[call, :, :])
                    at3 = at[:].rearrange("p (a n) -> p a n", a=16)

                    gat = iop.tile([P, SLOTS * 64], F32, tag="gat")
                    nc.gpsimd.dma_gather(
                        out_ap=gat[:].rearrange("p (n e) -> p n e", e=64),
                        in_ap=tab64,
                        idxs_ap=qt[:],
                        num_idxs=NIDX_CALL,
                        num_idxs_reg=NIDX_CALL,
                        elem_size=64,
                        single_packet=False,
                        queue_num=call % N_QUEUES,
                    )
                    g3 = gat[:].rearrange("p (n e) -> p n e", e=64)

                    # one-hot extraction: plane a overwrites the slots
                    # whose 16B record sits at sub-offset a of the super-row
                    ext = cmp.tile([P, SLOTS * 4], F32, tag="ext")
                    extv = ext[:].rearrange("p (n e) -> p n e", e=4)
                    for a in range(16):
                        nc.vector.add_instruction(
                            mybir.InstCopyPredicated(
                                name=nc.get_next_instruction_name(),
                                ins=[
                                    nc.vector.lower_ap(
                                        at3[:, a, :].unsqueeze(2)
                                        .broadcast_to((P, SLOTS, 4)), opt=False),
                                    nc.vector.lower_ap(
                                        g3[:, :, 4 * a:4 * a + 4], opt=False),
                                ],
                                outs=[nc.vector.lower_ap(extv, opt=False)],
                            ))

                    # slot n = g_sub*15 + k ; component views [P, B, K]
                    def comp(cidx):
                        return ext[:].rearrange(
                            "p (g k e) -> p g k e", k=K, e=4)[:, :, :, cidx:cidx + 1
                                                             ].squeeze(3)

                    def cb(cidx):
                        return pt3[:, gcol, cidx:cidx + 1].broadcast_to((P, B, K))

                    dx = cmp.tile([P, SLOTS], F32, tag="dx")
                    dx3 = dx[:].rearrange("p (g k) -> p g k", k=K)
                    dy = cmp.tile([P, SLOTS], F32, tag="dy")
                    dy3 = dy[:].rearrange("p (g k) -> p g k", k=K)
                    dz = cmp.tile([P, SLOTS], F32, tag="dz")
                    dz3 = dz[:].rearrange("p (g k) -> p g k", k=K)
                    d2 = cmp.tile([P, SLOTS], F32, tag="d2")
                    d23 = d2[:].rearrange("p (g k) -> p g k", k=K)

                    nc.vector.tensor_tensor(dx3, comp(0), cb(0), OP.subtract)
                    nc.vector.tensor_tensor(dy3, comp(1), cb(1), OP.subtract)
                    nc.vector.tensor_tensor(dz3, comp(2), cb(2), OP.subtract)
                    nc.vector.tensor_tensor(d23, dx3, dx3, OP.mult)
                    nc.vector.tensor_tensor(dy3, dy3, dy3, OP.mult)
                    nc.vector.tensor_tensor(d23, d23, dy3, OP.add)
                    nc.vector.tensor_tensor(dz3, dz3, dz3, OP.mult)
                    nc.vector.tensor_tensor(d23, d23, dz3, OP.add)

                    neg = cmp.tile([P, SLOTS], F32, tag="neg")
                    neg3 = neg[:].rearrange("p (g k) -> p g k", k=K)
                    nc.vector.tensor_tensor(
                        neg3, comp(3),
                        cls[:, gcol].unsqueeze(2).broadcast_to((P, B, K)),
                        OP.not_equal)

                    mn = cmp.tile([P, B], F32, tag="mn")
                    nc.vector.tensor_reduce(
                        mn[:].unsqueeze(2), d23, AX.X, op=OP.min)
                    nc.vector.tensor_tensor(
                        d23, d23, mn[:].unsqueeze(2).broadcast_to((P, B, K)),
                        OP.subtract)
                    e = cmp.tile([P, SLOTS], F32, tag="e")
                    nc.scalar.activation(
                        e[:], d2[:], mybir.ActivationFunctionType.Exp,
                        scale=-float(CCBETA))
                    e3 = e[:].rearrange("p (g k) -> p g k", k=K)

                    den = cmp.tile([P, B], F32, tag="den")
                    nc.vector.tensor_reduce(
                        den[:].unsqueeze(2), e3, AX.X, op=OP.add)
                    nc.vector.tensor_tensor(e3, e3, neg3, OP.mult)
                    num = cmp.tile([P, B], F32, tag="num")
                    nc.vector.tensor_reduce(
                        num[:].unsqueeze(2), e3, AX.X, op=OP.add)
                    rec = cmp.tile([P, B], F32, tag="rec")
                    nc.vector.reciprocal(rec[:], den[:])
                    nc.vector.tensor_tensor(out_sb[:, gcol], num[:], rec[:],
                                            OP.mult)

            nc.sync.dma_start(out=out.ap(), in_=out_sb[:])

    nc.compile()
    return nc


_CACHE = {}


def _get_nc(S=FULL_S, G=FULL_G):
    key = (S, G)
    if key not in _CACHE:
        _CACHE[key] = build(S, G)
    return _CACHE[key]


def make_in_maps(p, labels, neighbor_idx, S=FULL_S, G=FULL_G):
    """Shard + lay out inputs. Point n of core c (n in [0, SP)) lives at
    partition n%128, slot n//128; host-permuted arrays put it at row
    (n%128)*G + n//128. The permuted global table row of real point j is
    c*SP + (r%128)*G + r//128 with c = j//S, r = j%S."""
    SP = P * G
    CALLS = G // B
    n_total = p.shape[0]
    assert n_total == N_CORES * S
    in_maps = []
    for c in range(N_CORES):
        rows = slice(c * S, (c + 1) * S)
        lab_c = np.zeros((SP, C), dtype=np.float32)
        p_c = np.zeros((SP, 3), dtype=np.float32)
        # permuted layout: row p*G + g  <- point g*128 + p (if real)
        pos = np.arange(SP)                  # pos = p*G + g
        pp, gg = pos // G, pos % G
        n_of_pos = gg * P + pp               # point id at this row
        valid = n_of_pos < S
        src = np.where(valid, n_of_pos, 0)
        lab_c[valid] = labels[rows][src[valid]]
        p_c[valid] = p[rows][src[valid]]

        # pair index arrays
        j = neighbor_idx[rows].astype(np.int64)        # [S, K] global ids
        jc, jr = j // S, j % S
        jperm = jc * SP + (jr % P).astype(np.int64) * G + jr // P
        jp = np.zeros((SP, K), dtype=np.int64)
        jp[:S] = jperm
        # arrange by (g, p, k): block[g][p][k] -> list i = (gs*K+k)*P + p
        blk = jp.reshape(G, P, K)
        q16_all = np.empty((CALLS, P, NW_CALL), dtype=np.int16)
        a_all = np.empty((CALLS, P, 16 * SLOTS), dtype=np.int8)
        plane = np.arange(16).reshape(16, 1, 1)
        for call in range(CALLS):
            sub = blk[call * B:(call + 1) * B]            # [B, P, K]
            lst = sub.transpose(0, 2, 1).reshape(SLOTS, P)  # [t, p] ; i=t*P+p
            q16 = (lst >> 4).astype(np.int16)
            a8 = (lst & 15).astype(np.int8)
            wrapped = q16.reshape(-1).reshape(NW_CALL, 16).T  # [16, NW_CALL]
            q16_all[call] = np.tile(wrapped, (8, 1))
            onehot = (a8.T[None, :, :] == plane)             # [16, p, t]
            a_all[call] = onehot.transpose(1, 0, 2).reshape(
                P, 16 * SLOTS).astype(np.int8)
        in_maps.append({"labels": lab_c, "p3": p_c,
                        "qidx": q16_all, "aidx": a_all})
    return in_maps


def run(p, labels, neighbor_idx, S=FULL_S, G=FULL_G, trace=False):
    nc = _get_nc(S, G)
    in_maps = make_in_maps(p, labels, neighbor_idx, S, G)
    res = bass_utils.run_bass_kernel_spmd(
        nc, in_maps, core_ids=list(range(N_CORES)), trace=trace)
    outs = []
    for c in range(N_CORES):
        o2 = res.results[c]["out"].reshape(P, G)      # [p, g]
        outs.append(o2.T.reshape(-1)[:S])             # point n = g*128+p
    return np.concatenate(outs, axis=0).astype(np.float32), res


def kernel(p, labels, neighbor_idx):
    p = np.asarray(p, dtype=np.float32)
    labels = np.asarray(labels, dtype=np.float32)
    neighbor_idx = np.asarray(neighbor_idx)
    out, _ = run(p, labels, neighbor_idx)
    return out


# revision 14
# speedup vs baseline: 2.8307x; 1.0060x over previous
"""Trainium2 Bass kernel for the AmbiguityHead (retrieval_knn) problem.

Reference computation (per point i, K=15 neighbors j = nidx[i,k]):
    center_cls = argmax(labels[i])          (first occurrence on ties)
    neigh_cls  = argmax(labels[j])
    posmask    = center_cls == neigh_cls
    d2         = ||p[i] - p[j]||^2
    w          = softmax(-CCBETA * d2 over k)
    out[i]     = NU * sum_k w_k * (1 - posmask_k)

Strategy (8 NeuronCores, data-parallel over points; see spec sharding_hint):
  - Each core owns 62,500 points (padded to 63,488 = 128*496, laid out
    point-interleaved: point n -> partition n%128, slot n//128).
  - Phase 1: per-shard argmax class (encoded 13-argmax to keep jnp first-
    occurrence tie semantics), pack 16B records (x,y,z,clsenc), AllGather
    the packed table (8 x 63,488 rows) into pair-shared HBM.
  - Phase 2: the per-pair random gather uses the custom InstDMAGatherAnt
    ucode op: 256B super-rows (16 records) indexed by int16 j>>4 (the
    permuted table has 31,744 super-rows < 32768), issued over the 4 SWDGE
    queues. The 16B record is then extracted on-chip with a 4-level binary
    select tree keyed on bits of j&15, followed by the distance/softmax/
    compare reduction on the Vector/Scalar engines.
  - Host side does only sharding/layout: row permutations, index
    re-encoding (j -> [permuted row]>>4 / &15, int16 wrapped layout), and
    inverse permutation of the output. All arithmetic of the reference
    (argmax, distances, exp, reductions, compares) runs on device.
"""

import numpy as np

import concourse.bass as bass
import concourse.mybir as mybir
import concourse.tile as tile
from concourse import bacc
from concourse import bass_utils

F32 = mybir.dt.float32
I32 = mybir.dt.int32
I16 = mybir.dt.int16
I8 = mybir.dt.int8
AX = mybir.AxisListType
OP = mybir.AluOpType

N_CORES = 8
P = 128
C = 13
K = 15
CCBETA = 2.0
NU = 1.0

FULL_S = 500_000 // N_CORES   # real points per core
FULL_G = 496                  # point slots per partition (128*496 = 63488)
B = 4                         # point-groups (of 128) per gather call
NIDX_CALL = P * K * B         # 7680 indices per gather call
NW_CALL = NIDX_CALL // 16     # 480 idx per partition line (wrapped int16)
SLOTS = B * K                 # 60 pair slots per partition per call
DMA_SCRATCH = 65536
N_QUEUES = 4


def raw_dma_gather(nc, out_ap, in_ap, idxs_ap, num_idxs, elem_size, elem_step,
                   queue_num):
    """bass.dma_gather for elem_size*dtype % 256 != 0 (the ucode only needs
    the stride to be a 256B multiple for non-transpose HBM gathers; the
    bass-level assert is transpose-only in the ucode)."""
    eng = nc.gpsimd
    stride_bytes = elem_step * mybir.dt.size(in_ap.dtype)
    stride_bytes_256 = stride_bytes // 256
    assert stride_bytes % 256 == 0 and 0 < stride_bytes_256 < 256
    _in_ap = eng.lower_ap_dma(in_ap, for_custom_bir_dma=True)
    _idxs_ap = eng.lower_ap(idxs_ap)
    _out_ap = eng.lower_ap(out_ap)
    return eng.add_instruction(
        mybir.InstDMAGatherAnt(
            name=nc.get_next_instruction_name(),
            ins=[*_in_ap, _idxs_ap, eng.lower_val_access(eng.to_reg(num_idxs))],
            outs=[_out_ap],
            transpose=False,
            num_idxs=num_idxs,
            elem_size=elem_size,
            stride_bytes_256=stride_bytes_256,
            gen_mode=0,
            single_packet=False,
            queue_num=queue_num,
            sbuf_tokens_per_rank=0,
            sbuf_free_dim_per_rank=0,
            sbuf_free_dim_pad_per_rank=0,
            sbuf_byte_offset=0,
        ))


def select3(nc, out, mask, on_true, on_false):
    """nc.vector.select with un-merged APs so all three operands keep the
    same 3D shape (the stock path flattens only the contiguous out AP)."""
    v = nc.vector
    v.tensor_copy(out, on_false)
    return v.add_instruction(
        mybir.InstCopyPredicated(
            name=nc.get_next_instruction_name(),
            ins=[v.lower_ap(mask, opt=False), v.lower_ap(on_true, opt=False)],
            outs=[v.lower_ap(out, opt=False)],
        ))


def build(S=FULL_S, G=FULL_G):
    SP = P * G
    assert SP >= S and G % B == 0
    CALLS = G // B
    NTAB = N_CORES * SP          # permuted global table rows
    assert NTAB // 16 <= 32767   # int16 super-row indices

    nc = bacc.Bacc("TRN2", target_bir_lowering=False, debug=False,
                   num_devices=N_CORES, dynamic_dma_scratch_size=DMA_SCRATCH,
                   num_swdge_queues=N_QUEUES)

    # host-permuted inputs: row p*G+g holds point g*128+p of this core
    labels = nc.dram_tensor("labels", [SP, C], F32, kind="ExternalInput")
    p3 = nc.dram_tensor("p3", [SP, 3], F32, kind="ExternalInput")
    # per call: wrapped int16 super-row idxs [128, NW_CALL] and 16 one-hot
    # int8 mask planes (plane a marks slots whose record sits at sub-offset a)
    qidx = nc.dram_tensor("qidx", [CALLS, P, NW_CALL], I16, kind="ExternalInput")
    aidx = nc.dram_tensor("aidx", [CALLS, P, 16 * SLOTS], I8, kind="ExternalInput")
    out = nc.dram_tensor("out", [P, G], F32, kind="ExternalOutput")

    tshard = nc.dram_tensor("tshard", [SP, 4], F32)
    table_b = nc.dram_tensor("table_b", [NTAB, 4], F32, addr_space="Shared")
    tab64 = table_b.ap().rearrange("(q s) c -> q (s c)", s=16)  # [NTAB/16, 64]

    with tile.TileContext(nc) as tc:
        with tc.tile_pool(name="pers", bufs=1) as pers:
            # ---------------- phase 1: packed class/coord table ----------------
            with tc.tile_pool(name="ph1", bufs=1) as ph1:
                lab = ph1.tile([P, G * C], F32)
                nc.sync.dma_start(
                    out=lab[:], in_=labels.ap().rearrange("(p g) c -> p (g c)", p=P))
                lab3 = lab[:].rearrange("p (g c) -> p g c", c=C)

                m = ph1.tile([P, G], F32)
                nc.vector.tensor_reduce(m[:].unsqueeze(2), lab3, AX.X, op=OP.max)

                revio_i = ph1.tile([P, C], I32)
                nc.gpsimd.iota(revio_i[:], pattern=[[-1, C]], base=C,
                               channel_multiplier=0)
                revio = ph1.tile([P, C], F32)
                nc.vector.tensor_copy(revio[:], revio_i[:])

                eq = ph1.tile([P, G * C], F32)
                eq3 = eq[:].rearrange("p (g c) -> p g c", c=C)
                nc.vector.tensor_tensor(
                    eq3, lab3, m[:].unsqueeze(2).broadcast_to((P, G, C)),
                    OP.is_equal)
                nc.vector.tensor_tensor(
                    eq3, eq3, revio[:].unsqueeze(1).broadcast_to((P, G, C)),
                    OP.mult)
                cls = pers.tile([P, G], F32)
                nc.vector.tensor_reduce(cls[:].unsqueeze(2), eq3, AX.X, op=OP.max)

                pt = pers.tile([P, G * 3], F32)
                nc.sync.dma_start(
                    out=pt[:], in_=p3.ap().rearrange("(p g) c -> p (g c)", p=P))
                pt3 = pt[:].rearrange("p (g c) -> p g c", c=3)

                pk = ph1.tile([P, G * 4], F32)
                pk3 = pk[:].rearrange("p (g c) -> p g c", c=4)
                nc.vector.tensor_copy(pk3[:, :, 0:3], pt3)
                nc.vector.tensor_copy(pk3[:, :, 3:4], cls[:].unsqueeze(2))

                nc.sync.dma_start(
                    out=tshard.ap().rearrange("(p g) c -> p (g c)", p=P),
                    in_=pk[:])

                nc.gpsimd.collective_compute(
                    "AllGather", OP.bypass,
                    replica_groups=[list(range(N_CORES))],
                    ins=[tshard.ap().opt()],
                    outs=[table_b.ap().opt()],
                )

            # ---------------- phase 2: gather + extract + reduce --------------
            out_sb = pers.tile([P, G], F32)
            with tc.tile_pool(name="io", bufs=7) as iop, \
                 tc.tile_pool(name="cmp", bufs=2) as cmp:
                for call in range(CALLS):
                    gcol = slice(call * B, (call + 1) * B)
                    qt = iop.tile([P, NW_CALL], I16, tag="qt")
                    nc.sync.dma_start(out=qt[:], in_=qidx[call, :, :])
                    at = iop.tile([P, 16 * SLOTS], I8, tag="at")
                    nc.sync.dma_start(out=at[:], in_=aidx[call, :, :])
                    at3 = at[:].rearrange("p (a n) -> p a n", a=16)

                    gat = iop.tile([P, SLOTS * 64], F32, tag="gat")
                    nc.gpsimd.dma_gather(
                        out_ap=gat[:].rearrange("p (n e) -> p n e", e=64),
                        in_ap=tab64,
                        idxs_ap=qt[:],
                        num_idxs=NIDX_CALL,
                        num_idxs_reg=NIDX_CALL,
                        elem_size=64,
                        single_packet=False,
                        queue_num=call % N_QUEUES,
                    )
                    g3 = gat[:].rearrange("p (n e) -> p n e", e=64)

                    # one-hot extraction: plane a overwrites the slots
                    # whose 16B record sits at sub-offset a of the super-row
                    ext = cmp.tile([P, SLOTS * 4], F32, tag="ext")
                    extv = ext[:].rearrange("p (n e) -> p n e", e=4)
                    for a in range(16):
                        nc.vector.add_instruction(
                            mybir.InstCopyPredicated(
                                name=nc.get_next_instruction_name(),
                                ins=[
                                    nc.vector.lower_ap(
                                        at3[:, a, :].unsqueeze(2)
                                        .broadcast_to((P, SLOTS, 4)), opt=False),
                                    nc.vector.lower_ap(
                                        g3[:, :, 4 * a:4 * a + 4], opt=False),
                                ],
                                outs=[nc.vector.lower_ap(extv, opt=False)],
                            ))

                    # slot n = g_sub*15 + k ; component views [P, B, K]
                    def comp(cidx):
                        return ext[:].rearrange(
                            "p (g k e) -> p g k e", k=K, e=4)[:, :, :, cidx:cidx + 1
                                                             ].squeeze(3)

                    def cb(cidx):
                        return pt3[:, gcol, cidx:cidx + 1].broadcast_to((P, B, K))

                    dx = cmp.tile([P, SLOTS], F32, tag="dx")
                    dx3 = dx[:].rearrange("p (g k) -> p g k", k=K)
                    dy = cmp.tile([P, SLOTS], F32, tag="dy")
                    dy3 = dy[:].rearrange("p (g k) -> p g k", k=K)
                    dz = cmp.tile([P, SLOTS], F32, tag="dz")
                    dz3 = dz[:].rearrange("p (g k) -> p g k", k=K)
                    d2 = cmp.tile([P, SLOTS], F32, tag="d2")
                    d23 = d2[:].rearrange("p (g k) -> p g k", k=K)

                    nc.vector.tensor_tensor(dx3, comp(0), cb(0), OP.subtract)
                    nc.vector.tensor_tensor(dy3, comp(1), cb(1), OP.subtract)
                    nc.vector.tensor_tensor(dz3, comp(2), cb(2), OP.subtract)
                    nc.vector.tensor_tensor(d23, dx3, dx3, OP.mult)
                    nc.vector.tensor_tensor(dy3, dy3, dy3, OP.mult)
                    nc.vector.tensor_tensor(d23, d23, dy3, OP.add)
                    nc.vector.tensor_tensor(dz3, dz3, dz3, OP.mult)
                    nc.vector.tensor_tensor(d23, d23, dz3, OP.add)

                    neg = cmp.tile([P, SLOTS], F32, tag="neg")
                    neg3 = neg[:].rearrange("p (g k) -> p g k", k=K)
                    nc.vector.tensor_tensor(
                        neg3, comp(3),
                        cls[:, gcol].unsqueeze(2).broadcast_to((P, B, K)),
                        OP.not_equal)

                    mn = cmp.tile([P, B], F32, tag="mn")
                    nc.vector.tensor_reduce(
                        mn[:].unsqueeze(2), d23, AX.X, op=OP.min)
                    nc.vector.tensor_tensor(
                        d23, d23, mn[:].unsqueeze(2).broadcast_to((P, B, K)),
                        OP.subtract)
                    e = cmp.tile([P, SLOTS], F32, tag="e")
                    nc.scalar.activation(
                        e[:], d2[:], mybir.ActivationFunctionType.Exp,
                        scale=-float(CCBETA))
                    e3 = e[:].rearrange("p (g k) -> p g k", k=K)

                    den = cmp.tile([P, B], F32, tag="den")
                    nc.vector.tensor_reduce(
                        den[:].unsqueeze(2), e3, AX.X, op=OP.add)
                    nc.vector.tensor_tensor(e3, e3, neg3, OP.mult)
                    num = cmp.tile([P, B], F32, tag="num")
                    nc.vector.tensor_reduce(
                        num[:].unsqueeze(2), e3, AX.X, op=OP.add)
                    rec = cmp.tile([P, B], F32, tag="rec")
                    nc.vector.reciprocal(rec[:], den[:])
                    nc.vector.tensor_tensor(out_sb[:, gcol], num[:], rec[:],
                                            OP.mult)

            nc.sync.dma_start(out=out.ap(), in_=out_sb[:])

    nc.compile()
    return nc


_CACHE = {}


def _get_nc(S=FULL_S, G=FULL_G):
    key = (S, G)
    if key not in _CACHE:
        _CACHE[key] = build(S, G)
    return _CACHE[key]


def make_in_maps(p, labels, neighbor_idx, S=FULL_S, G=FULL_G):
    """Shard + lay out inputs. Point n of core c (n in [0, SP)) lives at
    partition n%128, slot n//128; host-permuted arrays put it at row
    (n%128)*G + n//128. The permuted global table row of real point j is
    c*SP + (r%128)*G + r//128 with c = j//S, r = j%S."""
    SP = P * G
    CALLS = G // B
    n_total = p.shape[0]
    assert n_total == N_CORES * S
    in_maps = []
    for c in range(N_CORES):
        rows = slice(c * S, (c + 1) * S)
        lab_c = np.zeros((SP, C), dtype=np.float32)
        p_c = np.zeros((SP, 3), dtype=np.float32)
        # permuted layout: row p*G + g  <- point g*128 + p (if real)
        pos = np.arange(SP)                  # pos = p*G + g
        pp, gg = pos // G, pos % G
        n_of_pos = gg * P + pp               # point id at this row
        valid = n_of_pos < S
        src = np.where(valid, n_of_pos, 0)
        lab_c[valid] = labels[rows][src[valid]]
        p_c[valid] = p[rows][src[valid]]

        # pair index arrays
        j = neighbor_idx[rows].astype(np.int64)        # [S, K] global ids
        jc, jr = j // S, j % S
        jperm = jc * SP + (jr % P).astype(np.int64) * G + jr // P
        jp = np.zeros((SP, K), dtype=np.int64)
        jp[:S] = jperm
        # arrange by (g, p, k): block[g][p][k] -> list i = (gs*K+k)*P + p
        blk = jp.reshape(G, P, K)
        q16_all = np.empty((CALLS, P, NW_CALL), dtype=np.int16)
        a_all = np.empty((CALLS, P, 16 * SLOTS), dtype=np.int8)
        plane = np.arange(16).reshape(16, 1, 1)
        for call in range(CALLS):
            sub = blk[call * B:(call + 1) * B]            # [B, P, K]
            lst = sub.transpose(0, 2, 1).reshape(SLOTS, P)  # [t, p] ; i=t*P+p
            q16 = (lst >> 4).astype(np.int16)
            a8 = (lst & 15).astype(np.int8)
            wrapped = q16.reshape(-1).reshape(NW_CALL, 16).T  # [16, NW_CALL]
            q16_all[call] = np.tile(wrapped, (8, 1))
            onehot = (a8.T[None, :, :] == plane)             # [16, p, t]
            a_all[call] = onehot.transpose(1, 0, 2).reshape(
                P, 16 * SLOTS).astype(np.int8)
        in_maps.append({"labels": lab_c, "p3": p_c,
                        "qidx": q16_all, "aidx": a_all})
    return in_maps


def run(p, labels, neighbor_idx, S=FULL_S, G=FULL_G, trace=False):
    nc = _get_nc(S, G)
    in_maps = make_in_maps(p, labels, neighbor_idx, S, G)
    res = bass_utils.run_bass_kernel_spmd(
        nc, in_maps, core_ids=list(range(N_CORES)), trace=trace)
    outs = []
    for c in range(N_CORES):
        o2 = res.results[c]["out"].reshape(P, G)      # [p, g]
        outs.append(o2.T.reshape(-1)[:S])             # point n = g*128+p
    return np.concatenate(outs, axis=0).astype(np.float32), res


def kernel(p, labels, neighbor_idx):
    p = np.asarray(p, dtype=np.float32)
    labels = np.asarray(labels, dtype=np.float32)
    neighbor_idx = np.asarray(neighbor_idx)
    out, _ = run(p, labels, neighbor_idx)
    return out


# revision 15
# speedup vs baseline: 3.2327x; 1.1420x over previous
"""Trainium2 Bass kernel for the AmbiguityHead (retrieval_knn) problem.

Reference computation (per point i, K=15 neighbors j = nidx[i,k]):
    center_cls = argmax(labels[i])          (first occurrence on ties)
    neigh_cls  = argmax(labels[j])
    posmask    = center_cls == neigh_cls
    d2         = ||p[i] - p[j]||^2
    w          = softmax(-CCBETA * d2 over k)
    out[i]     = NU * sum_k w_k * (1 - posmask_k)

Strategy (8 NeuronCores, data-parallel over points; see spec sharding_hint):
  - Each core owns 62,500 points (padded to 63,488 = 128*496, laid out
    point-interleaved: point n -> partition n%128, slot n//128).
  - Phase 1: per-shard argmax class (encoded 13-argmax to keep jnp first-
    occurrence tie semantics), pack 16B records (x,y,z,clsenc), AllGather
    the packed table (8 x 63,488 rows) into pair-shared HBM.
  - Phase 2: the per-pair random gather uses the custom InstDMAGatherAnt
    ucode op: 256B super-rows (16 records) indexed by int16 j>>4 (the
    permuted table has 31,744 super-rows < 32768), issued over the 4 SWDGE
    queues. The 16B record is then extracted on-chip with a 4-level binary
    select tree keyed on bits of j&15, followed by the distance/softmax/
    compare reduction on the Vector/Scalar engines.
  - Host side does only sharding/layout: row permutations, index
    re-encoding (j -> [permuted row]>>4 / &15, int16 wrapped layout), and
    inverse permutation of the output. All arithmetic of the reference
    (argmax, distances, exp, reductions, compares) runs on device.
"""

import numpy as np

import concourse.bass as bass
import concourse.mybir as mybir
import concourse.tile as tile
from concourse import bacc
from concourse import bass_utils

F32 = mybir.dt.float32
I32 = mybir.dt.int32
I16 = mybir.dt.int16
I8 = mybir.dt.int8
AX = mybir.AxisListType
OP = mybir.AluOpType

N_CORES = 8
P = 128
C = 13
K = 15
CCBETA = 2.0
NU = 1.0

FULL_S = 500_000 // N_CORES   # real points per core
FULL_G = 496                  # point slots per partition (128*496 = 63488)
B = 2                         # point-groups (of 128) per gather call
NIDX_CALL = P * K * B         # 7680 indices per gather call
NW_CALL = NIDX_CALL // 16     # 480 idx per partition line (wrapped int16)
SLOTS = B * K                 # 60 pair slots per partition per call
DMA_SCRATCH = 65536
N_QUEUES = 4


def raw_dma_gather(nc, out_ap, in_ap, idxs_ap, num_idxs, elem_size, elem_step,
                   queue_num):
    """bass.dma_gather for elem_size*dtype % 256 != 0 (the ucode only needs
    the stride to be a 256B multiple for non-transpose HBM gathers; the
    bass-level assert is transpose-only in the ucode)."""
    eng = nc.gpsimd
    stride_bytes = elem_step * mybir.dt.size(in_ap.dtype)
    stride_bytes_256 = stride_bytes // 256
    assert stride_bytes % 256 == 0 and 0 < stride_bytes_256 < 256
    _in_ap = eng.lower_ap_dma(in_ap, for_custom_bir_dma=True)
    _idxs_ap = eng.lower_ap(idxs_ap)
    _out_ap = eng.lower_ap(out_ap)
    return eng.add_instruction(
        mybir.InstDMAGatherAnt(
            name=nc.get_next_instruction_name(),
            ins=[*_in_ap, _idxs_ap, eng.lower_val_access(eng.to_reg(num_idxs))],
            outs=[_out_ap],
            transpose=False,
            num_idxs=num_idxs,
            elem_size=elem_size,
            stride_bytes_256=stride_bytes_256,
            gen_mode=0,
            single_packet=False,
            queue_num=queue_num,
            sbuf_tokens_per_rank=0,
            sbuf_free_dim_per_rank=0,
            sbuf_free_dim_pad_per_rank=0,
            sbuf_byte_offset=0,
        ))


def select3(nc, out, mask, on_true, on_false):
    """nc.vector.select with un-merged APs so all three operands keep the
    same 3D shape (the stock path flattens only the contiguous out AP)."""
    v = nc.vector
    v.tensor_copy(out, on_false)
    return v.add_instruction(
        mybir.InstCopyPredicated(
            name=nc.get_next_instruction_name(),
            ins=[v.lower_ap(mask, opt=False), v.lower_ap(on_true, opt=False)],
            outs=[v.lower_ap(out, opt=False)],
        ))


def build(S=FULL_S, G=FULL_G):
    SP = P * G
    assert SP >= S and G % B == 0
    CALLS = G // B
    NTAB = N_CORES * SP          # permuted global table rows
    assert NTAB // 16 <= 32767   # int16 super-row indices

    nc = bacc.Bacc("TRN2", target_bir_lowering=False, debug=False,
                   num_devices=N_CORES, dynamic_dma_scratch_size=DMA_SCRATCH,
                   num_swdge_queues=N_QUEUES)

    # host-permuted inputs: row p*G+g holds point g*128+p of this core
    labels = nc.dram_tensor("labels", [SP, C], F32, kind="ExternalInput")
    p3 = nc.dram_tensor("p3", [SP, 3], F32, kind="ExternalInput")
    # per call: wrapped int16 super-row idxs [128, NW_CALL] and 16 one-hot
    # int8 mask planes (plane a marks slots whose record sits at sub-offset a)
    qidx = nc.dram_tensor("qidx", [CALLS, P, NW_CALL], I16, kind="ExternalInput")
    aidx = nc.dram_tensor("aidx", [CALLS, P, 16 * SLOTS], I8, kind="ExternalInput")
    out = nc.dram_tensor("out", [P, G], F32, kind="ExternalOutput")

    tshard = nc.dram_tensor("tshard", [SP, 4], F32)
    table_b = nc.dram_tensor("table_b", [NTAB, 4], F32, addr_space="Shared")
    tab64 = table_b.ap().rearrange("(q s) c -> q (s c)", s=16)  # [NTAB/16, 64]

    with tile.TileContext(nc) as tc:
        with tc.tile_pool(name="pers", bufs=1) as pers:
            # ---------------- phase 1: packed class/coord table ----------------
            with tc.tile_pool(name="ph1", bufs=1) as ph1:
                lab = ph1.tile([P, G * C], F32)
                nc.sync.dma_start(
                    out=lab[:], in_=labels.ap().rearrange("(p g) c -> p (g c)", p=P))
                lab3 = lab[:].rearrange("p (g c) -> p g c", c=C)

                m = ph1.tile([P, G], F32)
                nc.vector.tensor_reduce(m[:].unsqueeze(2), lab3, AX.X, op=OP.max)

                revio_i = ph1.tile([P, C], I32)
                nc.gpsimd.iota(revio_i[:], pattern=[[-1, C]], base=C,
                               channel_multiplier=0)
                revio = ph1.tile([P, C], F32)
                nc.vector.tensor_copy(revio[:], revio_i[:])

                eq = ph1.tile([P, G * C], F32)
                eq3 = eq[:].rearrange("p (g c) -> p g c", c=C)
                nc.vector.tensor_tensor(
                    eq3, lab3, m[:].unsqueeze(2).broadcast_to((P, G, C)),
                    OP.is_equal)
                nc.vector.tensor_tensor(
                    eq3, eq3, revio[:].unsqueeze(1).broadcast_to((P, G, C)),
                    OP.mult)
                cls = pers.tile([P, G], F32)
                nc.vector.tensor_reduce(cls[:].unsqueeze(2), eq3, AX.X, op=OP.max)

                pt = pers.tile([P, G * 3], F32)
                nc.sync.dma_start(
                    out=pt[:], in_=p3.ap().rearrange("(p g) c -> p (g c)", p=P))
                pt3 = pt[:].rearrange("p (g c) -> p g c", c=3)

                pk = ph1.tile([P, G * 4], F32)
                pk3 = pk[:].rearrange("p (g c) -> p g c", c=4)
                nc.vector.tensor_copy(pk3[:, :, 0:3], pt3)
                nc.vector.tensor_copy(pk3[:, :, 3:4], cls[:].unsqueeze(2))

                nc.sync.dma_start(
                    out=tshard.ap().rearrange("(p g) c -> p (g c)", p=P),
                    in_=pk[:])

                nc.gpsimd.collective_compute(
                    "AllGather", OP.bypass,
                    replica_groups=[list(range(N_CORES))],
                    ins=[tshard.ap().opt()],
                    outs=[table_b.ap().opt()],
                )

            # ---------------- phase 2: gather + extract + reduce --------------
            out_sb = pers.tile([P, G], F32)
            with tc.tile_pool(name="io", bufs=8) as iop, \
                 tc.tile_pool(name="cmp", bufs=2) as cmp:
                for call in range(CALLS):
                    gcol = slice(call * B, (call + 1) * B)
                    qt = iop.tile([P, NW_CALL], I16, tag="qt")
                    nc.sync.dma_start(out=qt[:], in_=qidx[call, :, :])
                    at = iop.tile([P, 16 * SLOTS], I8, tag="at")
                    nc.sync.dma_start(out=at[:], in_=aidx[call, :, :])
                    at3 = at[:].rearrange("p (a n) -> p a n", a=16)

                    gat = iop.tile([P, SLOTS * 64], F32, tag="gat")
                    nc.gpsimd.dma_gather(
                        out_ap=gat[:].rearrange("p (n e) -> p n e", e=64),
                        in_ap=tab64,
                        idxs_ap=qt[:],
                        num_idxs=NIDX_CALL,
                        num_idxs_reg=NIDX_CALL,
                        elem_size=64,
                        single_packet=False,
                        queue_num=call % N_QUEUES,
                    )
                    g3 = gat[:].rearrange("p (n e) -> p n e", e=64)

                    # one-hot extraction: plane a overwrites the slots
                    # whose 16B record sits at sub-offset a of the super-row
                    ext = cmp.tile([P, SLOTS * 4], F32, tag="ext")
                    extv = ext[:].rearrange("p (n e) -> p n e", e=4)
                    for a in range(16):
                        nc.vector.add_instruction(
                            mybir.InstCopyPredicated(
                                name=nc.get_next_instruction_name(),
                                ins=[
                                    nc.vector.lower_ap(
                                        at3[:, a, :].unsqueeze(2)
                                        .broadcast_to((P, SLOTS, 4)), opt=False),
                                    nc.vector.lower_ap(
                                        g3[:, :, 4 * a:4 * a + 4], opt=False),
                                ],
                                outs=[nc.vector.lower_ap(extv, opt=False)],
                            ))

                    # slot n = g_sub*15 + k ; component views [P, B, K]
                    def comp(cidx):
                        return ext[:].rearrange(
                            "p (g k e) -> p g k e", k=K, e=4)[:, :, :, cidx:cidx + 1
                                                             ].squeeze(3)

                    def cb(cidx):
                        return pt3[:, gcol, cidx:cidx + 1].broadcast_to((P, B, K))

                    dx = cmp.tile([P, SLOTS], F32, tag="dx")
                    dx3 = dx[:].rearrange("p (g k) -> p g k", k=K)
                    dy = cmp.tile([P, SLOTS], F32, tag="dy")
                    dy3 = dy[:].rearrange("p (g k) -> p g k", k=K)
                    dz = cmp.tile([P, SLOTS], F32, tag="dz")
                    dz3 = dz[:].rearrange("p (g k) -> p g k", k=K)
                    d2 = cmp.tile([P, SLOTS], F32, tag="d2")
                    d23 = d2[:].rearrange("p (g k) -> p g k", k=K)

                    nc.vector.tensor_tensor(dx3, comp(0), cb(0), OP.subtract)
                    nc.vector.tensor_tensor(dy3, comp(1), cb(1), OP.subtract)
                    nc.vector.tensor_tensor(dz3, comp(2), cb(2), OP.subtract)
                    nc.vector.tensor_tensor(d23, dx3, dx3, OP.mult)
                    nc.vector.tensor_tensor(dy3, dy3, dy3, OP.mult)
                    nc.vector.tensor_tensor(d23, d23, dy3, OP.add)
                    nc.vector.tensor_tensor(dz3, dz3, dz3, OP.mult)
                    nc.vector.tensor_tensor(d23, d23, dz3, OP.add)

                    neg = cmp.tile([P, SLOTS], F32, tag="neg")
                    neg3 = neg[:].rearrange("p (g k) -> p g k", k=K)
                    nc.vector.tensor_tensor(
                        neg3, comp(3),
                        cls[:, gcol].unsqueeze(2).broadcast_to((P, B, K)),
                        OP.not_equal)

                    mn = cmp.tile([P, B], F32, tag="mn")
                    nc.vector.tensor_reduce(
                        mn[:].unsqueeze(2), d23, AX.X, op=OP.min)
                    nc.vector.tensor_tensor(
                        d23, d23, mn[:].unsqueeze(2).broadcast_to((P, B, K)),
                        OP.subtract)
                    e = cmp.tile([P, SLOTS], F32, tag="e")
                    nc.scalar.activation(
                        e[:], d2[:], mybir.ActivationFunctionType.Exp,
                        scale=-float(CCBETA))
                    e3 = e[:].rearrange("p (g k) -> p g k", k=K)

                    den = cmp.tile([P, B], F32, tag="den")
                    nc.vector.tensor_reduce(
                        den[:].unsqueeze(2), e3, AX.X, op=OP.add)
                    nc.vector.tensor_tensor(e3, e3, neg3, OP.mult)
                    num = cmp.tile([P, B], F32, tag="num")
                    nc.vector.tensor_reduce(
                        num[:].unsqueeze(2), e3, AX.X, op=OP.add)
                    rec = cmp.tile([P, B], F32, tag="rec")
                    nc.vector.reciprocal(rec[:], den[:])
                    nc.vector.tensor_tensor(out_sb[:, gcol], num[:], rec[:],
                                            OP.mult)

            nc.sync.dma_start(out=out.ap(), in_=out_sb[:])

    nc.compile()
    return nc


_CACHE = {}


def _get_nc(S=FULL_S, G=FULL_G):
    key = (S, G)
    if key not in _CACHE:
        _CACHE[key] = build(S, G)
    return _CACHE[key]


def make_in_maps(p, labels, neighbor_idx, S=FULL_S, G=FULL_G):
    """Shard + lay out inputs. Point n of core c (n in [0, SP)) lives at
    partition n%128, slot n//128; host-permuted arrays put it at row
    (n%128)*G + n//128. The permuted global table row of real point j is
    c*SP + (r%128)*G + r//128 with c = j//S, r = j%S."""
    SP = P * G
    CALLS = G // B
    n_total = p.shape[0]
    assert n_total == N_CORES * S
    in_maps = []
    for c in range(N_CORES):
        rows = slice(c * S, (c + 1) * S)
        lab_c = np.zeros((SP, C), dtype=np.float32)
        p_c = np.zeros((SP, 3), dtype=np.float32)
        # permuted layout: row p*G + g  <- point g*128 + p (if real)
        pos = np.arange(SP)                  # pos = p*G + g
        pp, gg = pos // G, pos % G
        n_of_pos = gg * P + pp               # point id at this row
        valid = n_of_pos < S
        src = np.where(valid, n_of_pos, 0)
        lab_c[valid] = labels[rows][src[valid]]
        p_c[valid] = p[rows][src[valid]]

        # pair index arrays
        j = neighbor_idx[rows].astype(np.int64)        # [S, K] global ids
        jc, jr = j // S, j % S
        jperm = jc * SP + (jr % P).astype(np.int64) * G + jr // P
        jp = np.zeros((SP, K), dtype=np.int64)
        jp[:S] = jperm
        # arrange by (g, p, k): block[g][p][k] -> list i = (gs*K+k)*P + p
        blk = jp.reshape(G, P, K)
        q16_all = np.empty((CALLS, P, NW_CALL), dtype=np.int16)
        a_all = np.empty((CALLS, P, 16 * SLOTS), dtype=np.int8)
        plane = np.arange(16).reshape(16, 1, 1)
        for call in range(CALLS):
            sub = blk[call * B:(call + 1) * B]            # [B, P, K]
            lst = sub.transpose(0, 2, 1).reshape(SLOTS, P)  # [t, p] ; i=t*P+p
            q16 = (lst >> 4).astype(np.int16)
            a8 = (lst & 15).astype(np.int8)
            wrapped = q16.reshape(-1).reshape(NW_CALL, 16).T  # [16, NW_CALL]
            q16_all[call] = np.tile(wrapped, (8, 1))
            onehot = (a8.T[None, :, :] == plane)             # [16, p, t]
            a_all[call] = onehot.transpose(1, 0, 2).reshape(
                P, 16 * SLOTS).astype(np.int8)
        in_maps.append({"labels": lab_c, "p3": p_c,
                        "qidx": q16_all, "aidx": a_all})
    return in_maps


def run(p, labels, neighbor_idx, S=FULL_S, G=FULL_G, trace=False):
    nc = _get_nc(S, G)
    in_maps = make_in_maps(p, labels, neighbor_idx, S, G)
    res = bass_utils.run_bass_kernel_spmd(
        nc, in_maps, core_ids=list(range(N_CORES)), trace=trace)
    outs = []
    for c in range(N_CORES):
        o2 = res.results[c]["out"].reshape(P, G)      # [p, g]
        outs.append(o2.T.reshape(-1)[:S])             # point n = g*128+p
    return np.concatenate(outs, axis=0).astype(np.float32), res


def kernel(p, labels, neighbor_idx):
    p = np.asarray(p, dtype=np.float32)
    labels = np.asarray(labels, dtype=np.float32)
    neighbor_idx = np.asarray(neighbor_idx)
    out, _ = run(p, labels, neighbor_idx)
    return out


# revision 16
# speedup vs baseline: 3.5768x; 1.1064x over previous
"""Trainium2 Bass kernel for the AmbiguityHead (retrieval_knn) problem.

Reference computation (per point i, K=15 neighbors j = nidx[i,k]):
    center_cls = argmax(labels[i])          (first occurrence on ties)
    neigh_cls  = argmax(labels[j])
    posmask    = center_cls == neigh_cls
    d2         = ||p[i] - p[j]||^2
    w          = softmax(-CCBETA * d2 over k)
    out[i]     = NU * sum_k w_k * (1 - posmask_k)

Strategy (8 NeuronCores, data-parallel over points; see spec sharding_hint):
  - Each core owns 62,500 points (padded to 63,488 = 128*496, laid out
    point-interleaved: point n -> partition n%128, slot n//128).
  - Phase 1: per-shard argmax class (encoded 13-argmax to keep jnp first-
    occurrence tie semantics), pack 16B records (x,y,z,clsenc), AllGather
    the packed table (8 x 63,488 rows) into pair-shared HBM.
  - Phase 2: the per-pair random gather uses the custom InstDMAGatherAnt
    ucode op: 256B super-rows (16 records) indexed by int16 j>>4 (the
    permuted table has 31,744 super-rows < 32768), issued over the 4 SWDGE
    queues. The 16B record is then extracted on-chip with a 4-level binary
    select tree keyed on bits of j&15, followed by the distance/softmax/
    compare reduction on the Vector/Scalar engines.
  - Host side does only sharding/layout: row permutations, index
    re-encoding (j -> [permuted row]>>4 / &15, int16 wrapped layout), and
    inverse permutation of the output. All arithmetic of the reference
    (argmax, distances, exp, reductions, compares) runs on device.
"""

import numpy as np

import concourse.bass as bass
import concourse.mybir as mybir
import concourse.tile as tile
from concourse import bacc
from concourse import bass_utils

F32 = mybir.dt.float32
I32 = mybir.dt.int32
I16 = mybir.dt.int16
I8 = mybir.dt.int8
AX = mybir.AxisListType
OP = mybir.AluOpType

N_CORES = 8
P = 128
C = 13
K = 15
CCBETA = 2.0
NU = 1.0

FULL_S = 500_000 // N_CORES   # real points per core
FULL_G = 496                  # point slots per partition (128*496 = 63488)
B = 2                         # point-groups (of 128) per gather call
NIDX_CALL = P * K * B         # 7680 indices per gather call
NW_CALL = NIDX_CALL // 16     # 480 idx per partition line (wrapped int16)
SLOTS = B * K                 # 60 pair slots per partition per call
DMA_SCRATCH = 65536
N_QUEUES = 4
R = 4                         # gather calls per compute group
RS = R * SLOTS                # pair slots per partition per group
BG = R * B                    # point-groups per compute group


def raw_dma_gather(nc, out_ap, in_ap, idxs_ap, num_idxs, elem_size, elem_step,
                   queue_num):
    """bass.dma_gather for elem_size*dtype % 256 != 0 (the ucode only needs
    the stride to be a 256B multiple for non-transpose HBM gathers; the
    bass-level assert is transpose-only in the ucode)."""
    eng = nc.gpsimd
    stride_bytes = elem_step * mybir.dt.size(in_ap.dtype)
    stride_bytes_256 = stride_bytes // 256
    assert stride_bytes % 256 == 0 and 0 < stride_bytes_256 < 256
    _in_ap = eng.lower_ap_dma(in_ap, for_custom_bir_dma=True)
    _idxs_ap = eng.lower_ap(idxs_ap)
    _out_ap = eng.lower_ap(out_ap)
    return eng.add_instruction(
        mybir.InstDMAGatherAnt(
            name=nc.get_next_instruction_name(),
            ins=[*_in_ap, _idxs_ap, eng.lower_val_access(eng.to_reg(num_idxs))],
            outs=[_out_ap],
            transpose=False,
            num_idxs=num_idxs,
            elem_size=elem_size,
            stride_bytes_256=stride_bytes_256,
            gen_mode=0,
            single_packet=False,
            queue_num=queue_num,
            sbuf_tokens_per_rank=0,
            sbuf_free_dim_per_rank=0,
            sbuf_free_dim_pad_per_rank=0,
            sbuf_byte_offset=0,
        ))


def select3(nc, out, mask, on_true, on_false):
    """nc.vector.select with un-merged APs so all three operands keep the
    same 3D shape (the stock path flattens only the contiguous out AP)."""
    v = nc.vector
    v.tensor_copy(out, on_false)
    return v.add_instruction(
        mybir.InstCopyPredicated(
            name=nc.get_next_instruction_name(),
            ins=[v.lower_ap(mask, opt=False), v.lower_ap(on_true, opt=False)],
            outs=[v.lower_ap(out, opt=False)],
        ))


def build(S=FULL_S, G=FULL_G):
    SP = P * G
    assert SP >= S and G % B == 0
    CALLS = G // B
    NTAB = N_CORES * SP          # permuted global table rows
    assert NTAB // 16 <= 32767   # int16 super-row indices

    nc = bacc.Bacc("TRN2", target_bir_lowering=False, debug=False,
                   num_devices=N_CORES, dynamic_dma_scratch_size=DMA_SCRATCH,
                   num_swdge_queues=N_QUEUES)

    # host-permuted inputs: row p*G+g holds point g*128+p of this core
    labels = nc.dram_tensor("labels", [SP, C], F32, kind="ExternalInput")
    p3 = nc.dram_tensor("p3", [SP, 3], F32, kind="ExternalInput")
    # per call: wrapped int16 super-row idxs [128, NW_CALL] and 16 one-hot
    # int8 mask planes (plane a marks slots whose record sits at sub-offset a)
    qidx = nc.dram_tensor("qidx", [CALLS, P, NW_CALL], I16, kind="ExternalInput")
    aidx = nc.dram_tensor("aidx", [CALLS // R, P, 16 * RS], I8, kind="ExternalInput")
    out = nc.dram_tensor("out", [P, G], F32, kind="ExternalOutput")

    tshard = nc.dram_tensor("tshard", [SP, 4], F32)
    table_b = nc.dram_tensor("table_b", [NTAB, 4], F32, addr_space="Shared")
    tab64 = table_b.ap().rearrange("(q s) c -> q (s c)", s=16)  # [NTAB/16, 64]

    with tile.TileContext(nc) as tc:
        with tc.tile_pool(name="pers", bufs=1) as pers:
            # ---------------- phase 1: packed class/coord table ----------------
            with tc.tile_pool(name="ph1", bufs=1) as ph1:
                lab = ph1.tile([P, G * C], F32)
                nc.sync.dma_start(
                    out=lab[:], in_=labels.ap().rearrange("(p g) c -> p (g c)", p=P))
                lab3 = lab[:].rearrange("p (g c) -> p g c", c=C)

                m = ph1.tile([P, G], F32)
                nc.vector.tensor_reduce(m[:].unsqueeze(2), lab3, AX.X, op=OP.max)

                revio_i = ph1.tile([P, C], I32)
                nc.gpsimd.iota(revio_i[:], pattern=[[-1, C]], base=C,
                               channel_multiplier=0)
                revio = ph1.tile([P, C], F32)
                nc.vector.tensor_copy(revio[:], revio_i[:])

                eq = ph1.tile([P, G * C], F32)
                eq3 = eq[:].rearrange("p (g c) -> p g c", c=C)
                nc.vector.tensor_tensor(
                    eq3, lab3, m[:].unsqueeze(2).broadcast_to((P, G, C)),
                    OP.is_equal)
                nc.vector.tensor_tensor(
                    eq3, eq3, revio[:].unsqueeze(1).broadcast_to((P, G, C)),
                    OP.mult)
                cls = pers.tile([P, G], F32)
                nc.vector.tensor_reduce(cls[:].unsqueeze(2), eq3, AX.X, op=OP.max)

                pt = pers.tile([P, G * 3], F32)
                nc.sync.dma_start(
                    out=pt[:], in_=p3.ap().rearrange("(p g) c -> p (g c)", p=P))
                pt3 = pt[:].rearrange("p (g c) -> p g c", c=3)

                pk = ph1.tile([P, G * 4], F32)
                pk3 = pk[:].rearrange("p (g c) -> p g c", c=4)
                nc.vector.tensor_copy(pk3[:, :, 0:3], pt3)
                nc.vector.tensor_copy(pk3[:, :, 3:4], cls[:].unsqueeze(2))

                nc.sync.dma_start(
                    out=tshard.ap().rearrange("(p g) c -> p (g c)", p=P),
                    in_=pk[:])

                nc.gpsimd.collective_compute(
                    "AllGather", OP.bypass,
                    replica_groups=[list(range(N_CORES))],
                    ins=[tshard.ap().opt()],
                    outs=[table_b.ap().opt()],
                )

            # ---------------- phase 2: gather + extract + reduce --------------
            out_sb = pers.tile([P, G], F32)
            with tc.tile_pool(name="io", bufs=3) as iop, \
                 tc.tile_pool(name="cmp", bufs=2) as cmp:
                for grp in range(CALLS // R):
                    gcol = slice(grp * BG, (grp + 1) * BG)
                    at = iop.tile([P, 16 * RS], I8, tag="at")
                    nc.sync.dma_start(out=at[:], in_=aidx[grp, :, :])
                    at3 = at[:].rearrange("p (a n) -> p a n", a=16)

                    gat = iop.tile([P, RS * 64], F32, tag="gat")
                    for q in range(R):
                        call = grp * R + q
                        qt = iop.tile([P, NW_CALL], I16, tag=f"qt{q}")
                        nc.sync.dma_start(out=qt[:], in_=qidx[call, :, :])
                        nc.gpsimd.dma_gather(
                            out_ap=gat[:, q * SLOTS * 64:(q + 1) * SLOTS * 64
                                       ].rearrange("p (n e) -> p n e", e=64),
                            in_ap=tab64,
                            idxs_ap=qt[:],
                            num_idxs=NIDX_CALL,
                            num_idxs_reg=NIDX_CALL,
                            elem_size=64,
                            single_packet=False,
                            queue_num=call % N_QUEUES,
                        )
                    g3 = gat[:].rearrange("p (n e) -> p n e", e=64)

                    # one-hot extraction: plane a overwrites the slots whose
                    # 16B record sits at sub-offset a of its super-row
                    ext = cmp.tile([P, RS * 4], F32, tag="ext")
                    extv = ext[:].rearrange("p (n e) -> p n e", e=4)
                    for a in range(16):
                        nc.vector.add_instruction(
                            mybir.InstCopyPredicated(
                                name=nc.get_next_instruction_name(),
                                ins=[
                                    nc.vector.lower_ap(
                                        at3[:, a, :].unsqueeze(2)
                                        .broadcast_to((P, RS, 4)), opt=False),
                                    nc.vector.lower_ap(
                                        g3[:, :, 4 * a:4 * a + 4], opt=False),
                                ],
                                outs=[nc.vector.lower_ap(extv, opt=False)],
                            ))

                    # slot n = g_sub*15 + k ; component views [P, BG, K]
                    def comp(cidx):
                        return ext[:].rearrange(
                            "p (g k e) -> p g k e", k=K, e=4)[:, :, :, cidx:cidx + 1
                                                             ].squeeze(3)

                    def cb(cidx):
                        return pt3[:, gcol, cidx:cidx + 1].broadcast_to((P, BG, K))

                    dx = cmp.tile([P, RS], F32, tag="dx")
                    dx3 = dx[:].rearrange("p (g k) -> p g k", k=K)
                    dy = cmp.tile([P, RS], F32, tag="dy")
                    dy3 = dy[:].rearrange("p (g k) -> p g k", k=K)
                    dz = cmp.tile([P, RS], F32, tag="dz")
                    dz3 = dz[:].rearrange("p (g k) -> p g k", k=K)
                    d2 = cmp.tile([P, RS], F32, tag="d2")
                    d23 = d2[:].rearrange("p (g k) -> p g k", k=K)

                    nc.vector.tensor_tensor(dx3, comp(0), cb(0), OP.subtract)
                    nc.vector.tensor_tensor(dy3, comp(1), cb(1), OP.subtract)
                    nc.vector.tensor_tensor(dz3, comp(2), cb(2), OP.subtract)
                    nc.vector.tensor_tensor(d23, dx3, dx3, OP.mult)
                    nc.vector.tensor_tensor(dy3, dy3, dy3, OP.mult)
                    nc.vector.tensor_tensor(d23, d23, dy3, OP.add)
                    nc.vector.tensor_tensor(dz3, dz3, dz3, OP.mult)
                    nc.vector.tensor_tensor(d23, d23, dz3, OP.add)

                    neg = cmp.tile([P, RS], F32, tag="neg")
                    neg3 = neg[:].rearrange("p (g k) -> p g k", k=K)
                    nc.vector.tensor_tensor(
                        neg3, comp(3),
                        cls[:, gcol].unsqueeze(2).broadcast_to((P, BG, K)),
                        OP.not_equal)

                    mn = cmp.tile([P, BG], F32, tag="mn")
                    nc.vector.tensor_reduce(
                        mn[:].unsqueeze(2), d23, AX.X, op=OP.min)
                    nc.vector.tensor_tensor(
                        d23, d23, mn[:].unsqueeze(2).broadcast_to((P, BG, K)),
                        OP.subtract)
                    e = cmp.tile([P, RS], F32, tag="e")
                    nc.scalar.activation(
                        e[:], d2[:], mybir.ActivationFunctionType.Exp,
                        scale=-float(CCBETA))
                    e3 = e[:].rearrange("p (g k) -> p g k", k=K)

                    den = cmp.tile([P, BG], F32, tag="den")
                    nc.vector.tensor_reduce(
                        den[:].unsqueeze(2), e3, AX.X, op=OP.add)
                    nc.vector.tensor_tensor(e3, e3, neg3, OP.mult)
                    num = cmp.tile([P, BG], F32, tag="num")
                    nc.vector.tensor_reduce(
                        num[:].unsqueeze(2), e3, AX.X, op=OP.add)
                    rec = cmp.tile([P, BG], F32, tag="rec")
                    nc.vector.reciprocal(rec[:], den[:])
                    nc.vector.tensor_tensor(out_sb[:, gcol], num[:], rec[:],
                                            OP.mult)

            nc.sync.dma_start(out=out.ap(), in_=out_sb[:])

    nc.compile()
    return nc


_CACHE = {}


def _get_nc(S=FULL_S, G=FULL_G):
    key = (S, G)
    if key not in _CACHE:
        _CACHE[key] = build(S, G)
    return _CACHE[key]


def make_in_maps(p, labels, neighbor_idx, S=FULL_S, G=FULL_G):
    """Shard + lay out inputs. Point n of core c (n in [0, SP)) lives at
    partition n%128, slot n//128; host-permuted arrays put it at row
    (n%128)*G + n//128. The permuted global table row of real point j is
    c*SP + (r%128)*G + r//128 with c = j//S, r = j%S."""
    SP = P * G
    CALLS = G // B
    n_total = p.shape[0]
    assert n_total == N_CORES * S
    in_maps = []
    for c in range(N_CORES):
        rows = slice(c * S, (c + 1) * S)
        lab_c = np.zeros((SP, C), dtype=np.float32)
        p_c = np.zeros((SP, 3), dtype=np.float32)
        # permuted layout: row p*G + g  <- point g*128 + p (if real)
        pos = np.arange(SP)                  # pos = p*G + g
        pp, gg = pos // G, pos % G
        n_of_pos = gg * P + pp               # point id at this row
        valid = n_of_pos < S
        src = np.where(valid, n_of_pos, 0)
        lab_c[valid] = labels[rows][src[valid]]
        p_c[valid] = p[rows][src[valid]]

        # pair index arrays
        j = neighbor_idx[rows].astype(np.int64)        # [S, K] global ids
        jc, jr = j // S, j % S
        jperm = jc * SP + (jr % P).astype(np.int64) * G + jr // P
        jp = np.zeros((SP, K), dtype=np.int64)
        jp[:S] = jperm
        # arrange by (g, p, k): block[g][p][k] -> list i = (gs*K+k)*P + p
        blk = jp.reshape(G, P, K)
        q16_all = np.empty((CALLS, P, NW_CALL), dtype=np.int16)
        a_all = np.empty((CALLS // R, P, 16 * RS), dtype=np.int8)
        plane = np.arange(16).reshape(16, 1, 1)
        a8_grp = np.empty((P, RS), dtype=np.int8)
        for call in range(CALLS):
            sub = blk[call * B:(call + 1) * B]            # [B, P, K]
            lst = sub.transpose(0, 2, 1).reshape(SLOTS, P)  # [t, p] ; i=t*P+p
            q16 = (lst >> 4).astype(np.int16)
            a8 = (lst & 15).astype(np.int8)
            wrapped = q16.reshape(-1).reshape(NW_CALL, 16).T  # [16, NW_CALL]
            q16_all[call] = np.tile(wrapped, (8, 1))
            q = call % R
            a8_grp[:, q * SLOTS:(q + 1) * SLOTS] = a8.T
            if q == R - 1:
                onehot = (a8_grp[None, :, :] == plane)       # [16, p, RS]
                a_all[call // R] = onehot.transpose(1, 0, 2).reshape(
                    P, 16 * RS).astype(np.int8)
        in_maps.append({"labels": lab_c, "p3": p_c,
                        "qidx": q16_all, "aidx": a_all})
    return in_maps


def run(p, labels, neighbor_idx, S=FULL_S, G=FULL_G, trace=False):
    nc = _get_nc(S, G)
    in_maps = make_in_maps(p, labels, neighbor_idx, S, G)
    res = bass_utils.run_bass_kernel_spmd(
        nc, in_maps, core_ids=list(range(N_CORES)), trace=trace)
    outs = []
    for c in range(N_CORES):
        o2 = res.results[c]["out"].reshape(P, G)      # [p, g]
        outs.append(o2.T.reshape(-1)[:S])             # point n = g*128+p
    return np.concatenate(outs, axis=0).astype(np.float32), res


def kernel(p, labels, neighbor_idx):
    p = np.asarray(p, dtype=np.float32)
    labels = np.asarray(labels, dtype=np.float32)
    neighbor_idx = np.asarray(neighbor_idx)
    out, _ = run(p, labels, neighbor_idx)
    return out
